# revision 15
# baseline (speedup 1.0000x reference)
"""Trainium2 Bass kernel: multi-head self-attention block (B=16, N=1024, C=768, H=12).

Data-parallel over batch: 8 NeuronCores x 2 batches each, no collectives.

Dataflow (per core, all-transposed activations; no on-chip transposes):
  host: xT = x_shard^T                                  [C, T]
  qkT  = W_qkv[:, :2C]^T-tiles @ xT   (per batch)       [2C, N]   (q^T | k^T)
  v'   = xT-tiles^T @ W_qkv[:, 2C:]  (+ ones col/head)  [N, H*(HD+1)]
  S^T  = k^T-slices^T @ q^T  (head pair packed in one   [128, 1024]
         2-bank PSUM tile: head A cols 0:512, B 512:)
  E    = exp(SCALE * S^T)     (ONE ScalarE op per step)
  U'   = v'^T @ E  (accum over k; row HD = softmax Z)   [HD+1, 512]
  aoT  = U'[:HD] * (1/Z broadcast)                      [C, N]
  y    = aoT-tiles^T @ W_proj + b                       [N, C]

Scheduling: one global software pipeline over (batch, head-pair, q-block,
k-tile) steps.  S(i+1) is emitted before U(i) so the PE never sits on the
exp latency; all projection work (v-phase, qk projections, out-proj of
batch 0) is chopped into 6-matmul "filler" groups pumped one-per-step into
2 spare PSUM banks, hiding it inside the Scalar-paced attention windows.
PSUM: S-ring 2x[128,1024] (8KB) + U-accum 2x[65,512] (4KB) + filler
2x[128,512] (4KB) = 16KB exactly.
"""

import sys
from collections import deque

for _p in ("/opt/trn_rl_repo", "/opt/pypackages"):
    if _p not in sys.path:
        sys.path.append(_p)

import numpy as np

B, N, C, H = 16, 1024, 768, 12
HD = C // H            # 64
SCALE = HD ** -0.5
NCORES = 8
BL = B // NCORES       # 2 batches per core
T = BL * N             # 2048 tokens per core

COMPUTE = "bf16"       # "bf16" | "f32" | "f32r"


def build_attention_nc(compute=COMPUTE, bl=BL, n=N, c=C, h=H):
    import concourse.bass as bass
    import concourse.tile as tile
    from concourse import bacc, mybir
    from contextlib import ExitStack

    hd = c // h
    t = bl * n
    scale = hd ** -0.5
    assert c % 128 == 0 and n % 512 == 0 and h % 2 == 0 and hd == 64
    CCH = c // 128      # contraction chunks over channels (6)
    NHP = h // 2        # head pairs (6)
    NQB = n // 512      # q-blocks per sequence (2)
    NKT = n // 128      # 128-wide k/token tiles per sequence (8)
    NXH = n // 512      # 512-wide x tiles per sequence (2)
    VW = hd + 1         # v' width per head (ones col at hd)
    PH = c // 2         # proj/v free-dim half (384) <= 1 PSUM bank

    FP32 = mybir.dt.float32
    SD = mybir.dt.bfloat16 if compute == "bf16" else FP32  # storage dtype

    def mm(ap):
        return ap.bitcast(mybir.dt.float32r) if compute == "f32r" else ap

    nc = bacc.Bacc("TRN2", target_bir_lowering=False, debug=False,
                   num_devices=NCORES)

    xT_d = nc.dram_tensor("xT", [c, t], SD, kind="ExternalInput").ap()
    wqkv_d = nc.dram_tensor("w_qkv", [c, 3 * c], SD, kind="ExternalInput").ap()
    wproj_d = nc.dram_tensor("w_proj", [c, c], SD, kind="ExternalInput").ap()
    bias_d = nc.dram_tensor("bias", [128, c], FP32, kind="ExternalInput").ap()
    out_d = nc.dram_tensor("out", [t, c], FP32, kind="ExternalOutput").ap()

    Exp = mybir.ActivationFunctionType.Exp

    with tile.TileContext(nc) as tc, ExitStack() as ctx:
        consts = ctx.enter_context(tc.tile_pool(name="consts", bufs=1))
        xp = ctx.enter_context(tc.tile_pool(name="xp", bufs=2))
        qkp = ctx.enter_context(tc.tile_pool(name="qkp", bufs=2))
        vp = ctx.enter_context(tc.tile_pool(name="vp", bufs=2))
        ep = ctx.enter_context(tc.tile_pool(name="ep", bufs=4))
        aop = ctx.enter_context(tc.tile_pool(name="aop", bufs=2))
        smp = ctx.enter_context(tc.tile_pool(name="smp", bufs=2))
        yp = ctx.enter_context(tc.tile_pool(name="yp", bufs=4))
        # PSUM: 16KB/partition total. s: 2x4KB, u: 2x2KB, p: 2x2KB.
        ps_s = ctx.enter_context(tc.tile_pool(name="ps_s", bufs=2, space="PSUM"))
        ps_u = ctx.enter_context(tc.tile_pool(name="ps_u", bufs=2, space="PSUM"))
        ps_p = ctx.enter_context(tc.tile_pool(name="ps_p", bufs=2, space="PSUM"))

        # ---------------- DMA loads (ordered for the pipeline ramp) -------
        # The load path is one serialized queue at ~320GB/s, so order by
        # first-use: wv + x(b0,xh0) (v-phase), then just the two 128-col
        # slivers of wqk that hp0 needs, then x(b0,xh1), then the rest.
        # Dependency tracking is tile-granular: hp0's qk weights go in their
        # own small tiles so the first attention window never waits on the
        # bulk wqk load.
        wv_sb = []
        wqk_sb = []
        wq0_sb = []
        wk0_sb = []
        xT_all = [[[None] * NXH for _ in range(CCH)] for _ in range(bl)]
        for cc in range(CCH):
            wv = consts.tile([128, c], SD, tag=f"wv{cc}")
            nc.sync.dma_start(out=wv, in_=wqkv_d[cc * 128:(cc + 1) * 128,
                                                 2 * c:3 * c])
            wv_sb.append(wv)
            xt = xp.tile([128, 512], SD, tag=f"x{cc}_0", name=f"x_b0c{cc}h0")
            nc.sync.dma_start(out=xt, in_=xT_d[cc * 128:(cc + 1) * 128, 0:512])
            xT_all[0][cc][0] = xt
            wq0 = consts.tile([128, 128], SD, tag=f"wq0_{cc}")
            nc.sync.dma_start(out=wq0, in_=wqkv_d[cc * 128:(cc + 1) * 128,
                                                  0:128])
            wq0_sb.append(wq0)
            wk0 = consts.tile([128, 128], SD, tag=f"wk0_{cc}")
            nc.sync.dma_start(out=wk0, in_=wqkv_d[cc * 128:(cc + 1) * 128,
                                                  c:c + 128])
            wk0_sb.append(wk0)
        # 2) x(b0) second half (qk qn1 + k-tiles 4..7 of the first window).
        for cc in range(CCH):
            xt = xp.tile([128, 512], SD, tag=f"x{cc}_1", name=f"x_b0c{cc}h1")
            nc.sync.dma_start(out=xt, in_=xT_d[cc * 128:(cc + 1) * 128,
                                              512:1024])
            xT_all[0][cc][1] = xt
        # 3) bulk wqk (used from hp1 on).
        for cc in range(CCH):
            w1 = consts.tile([128, 2 * c], SD, tag=f"wqkv{cc}")
            nc.sync.dma_start(out=w1, in_=wqkv_d[cc * 128:(cc + 1) * 128,
                                                 0:2 * c])
            wqk_sb.append(w1)
        # 4) x(b1), wproj, bias: needed mid-program.
        for cc in range(CCH):
            for xh in range(NXH):
                xt = xp.tile([128, 512], SD, tag=f"x{cc}_{xh}",
                             name=f"x_b1c{cc}h{xh}")
                nc.sync.dma_start(
                    out=xt, in_=xT_d[cc * 128:(cc + 1) * 128,
                                     n + xh * 512:n + (xh + 1) * 512])
                xT_all[1][cc][xh] = xt
        wproj_sb = []
        for cc in range(CCH):
            w2 = consts.tile([128, c], SD, tag=f"wproj{cc}")
            nc.sync.dma_start(out=w2, in_=wproj_d[cc * 128:(cc + 1) * 128, :])
            wproj_sb.append(w2)
        bias_sb = consts.tile([128, c], FP32, tag="bias")
        nc.sync.dma_start(out=bias_sb, in_=bias_d)

        # ---------------- group emitters ---------------------------------
        # Each emits 6 accumulating matmuls into a PSUM tile from pool/tag,
        # then a high-priority DVE evacuation.

        v_all = [[None] * NKT for _ in range(bl)]

        def v_tile_of(b, tt):
            if v_all[b][tt] is None:
                vt = vp.tile([128, h * VW], SD, tag=f"v{tt}", name=f"v_b{b}t{tt}")
                ones_view = vt[:, :].rearrange("p (hh w) -> p hh w", hh=h)[:, :, hd:hd + 1]
                nc.gpsimd.memset(ones_view, 1.0)
                v_all[b][tt] = vt
            return v_all[b][tt]

        HCC = CCH // 2   # matmuls per filler half-group

        def v_parts(b, tt, half, pool, ptag):
            st = {}

            def part(lo, hi):
                def go():
                    vt = v_tile_of(b, tt)
                    if lo == 0:
                        st["ps"] = pool.tile([128, PH], FP32, tag=ptag,
                                             name=f"vps_b{b}t{tt}f{half}")
                    ps = st["ps"]
                    xh, tl = tt // 4, tt % 4
                    for cc in range(lo, hi):
                        nc.tensor.matmul(
                            ps,
                            lhsT=mm(xT_all[b][cc][xh][:, tl * 128:(tl + 1) * 128]),
                            rhs=mm(wv_sb[cc][:, half * PH:(half + 1) * PH]),
                            start=(cc == 0), stop=(cc == CCH - 1))
                    if hi == CCH:
                        nheads = PH // hd
                        dst = vt[:, half * nheads * VW:(half + 1) * nheads * VW
                                 ].rearrange("p (hh w) -> p hh w",
                                             hh=nheads)[:, :, 0:hd]
                        srcv = ps[:].rearrange("p (hh w) -> p hh w", hh=nheads)
                        with tc.high_priority(offset=300):
                            nc.vector.tensor_copy(dst, srcv)
                return go
            return [part(0, HCC), part(HCC, CCH)]

        def emit_v_group(b, tt, half, pool, ptag):
            for p in v_parts(b, tt, half, pool, ptag):
                p()

        qk_tiles = {}

        def qk_dst(b, hp, which, qn):
            key = (b, hp, which, qn)
            if key not in qk_tiles:
                qk_tiles[key] = qkp.tile([128, 512], SD, tag=f"{which}{qn}",
                                         name=f"{which}{qn}_b{b}hp{hp}")
            return qk_tiles[key]

        def qk_parts(b, hp, which, qn, pool, ptag):
            st = {}

            def part(lo, hi):
                def go():
                    dst = qk_dst(b, hp, which, qn)
                    if hp == 0:
                        w_of = (lambda cc: wq0_sb[cc][:, 0:128]) \
                            if which == "qt" else \
                            (lambda cc: wk0_sb[cc][:, 0:128])
                    else:
                        fbase = hp * 128 if which == "qt" else c + hp * 128
                        w_of = lambda cc: wqk_sb[cc][:, fbase:fbase + 128]
                    if lo == 0:
                        st["ps"] = pool.tile([128, 512], FP32, tag=ptag,
                                             name=f"qkps_{which}_b{b}hp{hp}q{qn}")
                    ps = st["ps"]
                    for cc in range(lo, hi):
                        nc.tensor.matmul(
                            ps,
                            lhsT=mm(w_of(cc)),
                            rhs=mm(xT_all[b][cc][qn]),
                            start=(cc == 0), stop=(cc == CCH - 1))
                    if hi == CCH:
                        with tc.high_priority(offset=300):
                            nc.vector.tensor_copy(dst, ps)
                return go
            return [part(0, HCC), part(HCC, CCH)]

        def emit_qk_group(b, hp, which, qn, pool, ptag):
            for p in qk_parts(b, hp, which, qn, pool, ptag):
                p()

        ao_tiles = {}

        def out_parts(b, tt, half, pool, ptag):
            st = {}

            def part(lo, hi):
                def go():
                    if lo == 0:
                        st["ps"] = pool.tile([128, PH], FP32, tag=ptag,
                                             name=f"yps_b{b}t{tt}f{half}")
                    ps = st["ps"]
                    for cc in range(lo, hi):
                        nc.tensor.matmul(
                            ps,
                            lhsT=mm(ao_tiles[(b, cc)][:, tt * 128:(tt + 1) * 128]),
                            rhs=mm(wproj_sb[cc][:, half * PH:(half + 1) * PH]),
                            start=(cc == 0), stop=(cc == CCH - 1))
                    if hi == CCH:
                        yt = yp.tile([128, PH], FP32, tag="y",
                                     name=f"y_b{b}t{tt}f{half}")
                        with tc.high_priority(offset=300):
                            nc.vector.tensor_add(
                                yt, ps, bias_sb[:, half * PH:(half + 1) * PH])
                        nc.sync.dma_start(
                            out=out_d[b * n + tt * 128:b * n + (tt + 1) * 128,
                                      half * PH:(half + 1) * PH],
                            in_=yt)
                return go
            return [part(0, HCC), part(HCC, CCH)]

        def emit_out_group(b, tt, half, pool, ptag):
            for p in out_parts(b, tt, half, pool, ptag):
                p()

        # ---------------- filler pump ------------------------------------
        fillers = deque()

        def pump(k=1):
            for _ in range(k):
                if fillers:
                    fillers.popleft()()

        # ---------------- head phase -------------------------------------
        # Just enough to start attention: v(b0, tt0) both halves + qk(b0,0).
        # Rotate over the (still idle) u and p slots for double buffering.
        head_rot = [("u", ps_u), ("p", ps_p)]
        head_groups = [lambda pool, ptag: emit_v_group(0, 0, 0, pool, ptag),
                       lambda pool, ptag: emit_v_group(0, 0, 1, pool, ptag)]
        for qn in range(NQB):
            for which in ("qt", "kt"):
                head_groups.append(
                    (lambda which=which, qn=qn: lambda pool, ptag:
                     emit_qk_group(0, 0, which, qn, pool, ptag))())
        for i, g in enumerate(head_groups):
            ptag, pool = head_rot[i % 2]
            g(pool, ptag)

        # ---------------- window filler schedule --------------------------
        # Each entry is one pump event: either a whole v group (JIT, hp0) or
        # a 3-matmul half-group. 16 pump slots per window (one per kt-step).
        # v halves: half 0 feeds head pairs 0..2, half 1 feeds 3..5.
        def qk_f(b, hp):
            out = []
            for qn in range(NQB):
                for which in ("qt", "kt"):
                    out.extend(qk_parts(b, hp, which, qn, ps_p, "p"))
            return out

        def v_f(b, tts, half):
            # whole groups: one LDW stall amortized over 6 matmuls
            return [(lambda tt=tt: emit_v_group(b, tt, half, ps_p, "p"))
                    for tt in tts]

        def window_fillers(b, hp):
            if b == 0:
                if hp == 0:
                    # JIT v(b0) half-0 tiles tt1..7 as whole groups: tt_k at
                    # pump slot k-1 lands just before U(kt=k-1).
                    return [(lambda tt=tt: emit_v_group(0, tt, 0, ps_p, "p"))
                            for tt in range(1, NKT)] + qk_f(0, 1)
                if hp == 1:
                    return qk_f(0, 2) + v_f(0, range(0, 4), 1)
                if hp == 2:
                    return qk_f(0, 3) + v_f(0, range(4, NKT), 1)
                if hp == 3:
                    return qk_f(0, 4) + v_f(1, range(0, 4), 0)
                if hp == 4:
                    return qk_f(0, 5) + v_f(1, range(4, NKT), 0)
                return qk_f(1, 0)
            else:
                all_out = [(tt, half) for tt in range(NKT) for half in range(2)]
                out = []
                if hp < NHP - 1:
                    out += qk_f(1, hp + 1)
                if hp == 0:
                    out += v_f(1, range(0, 4), 1)
                elif hp == 1:
                    out += v_f(1, range(4, NKT), 1)
                else:
                    for tt, half in all_out[(hp - 2) * 4:(hp - 1) * 4]:
                        out.append(lambda tt=tt, half=half:
                                   emit_out_group(0, tt, half, ps_p, "p"))
                return out

        # ---------------- attention: global software pipeline -------------
        steps = [(b, hp, qblk, kt)
                 for b in range(bl)
                 for hp in range(NHP)
                 for qblk in range(NQB)
                 for kt in range(NKT)]

        u_ps = {}     # (qblk % 2, head) -> psum accum tile (ring by alloc)
        et_by_step = {}
        ao_cur = {}   # (b, hp) -> ao tile

        def emit_S(i):
            b, hp, qblk, kt = steps[i]
            if qblk == 0 and kt == 0:
                ao_cur[(b, hp)] = aop.tile([128, n], SD, tag=f"ao{hp}",
                                           name=f"ao_b{b}hp{hp}")
                fillers.extend(window_fillers(b, hp))
            if kt == 0:
                for head in range(2):
                    u_ps[(i, head)] = ps_u.tile(
                        [VW, 512], FP32, tag="u",
                        name=f"u_b{b}hp{hp}q{qblk}h{head}")
            qt_t = qk_dst(b, hp, "qt", qblk)
            kt_t = qk_dst(b, hp, "kt", kt // 4)
            ko = (kt % 4) * 128
            sps = ps_s.tile([128, 2 * 512], FP32, tag="s",
                            name=f"s_b{b}hp{hp}q{qblk}k{kt}")
            for head in range(2):
                p0 = head * 64
                nc.tensor.matmul(
                    sps[:, head * 512:(head + 1) * 512],
                    lhsT=mm(kt_t[p0:p0 + 64, ko:ko + 128]),
                    rhs=mm(qt_t[p0:p0 + 64, :]),
                    start=True, stop=True)
            et = ep.tile([128, 2 * 512], SD, tag="e",
                         name=f"e_b{b}hp{hp}q{qblk}k{kt}")
            nc.scalar.activation(et, sps, Exp, scale=scale)
            et_by_step[i] = et

        def emit_U(i):
            b, hp, qblk, kt = steps[i]
            et = et_by_step.pop(i)
            base = i - kt
            for head in range(2):
                hh = 2 * hp + head
                nc.tensor.matmul(
                    u_ps[(base, head)],
                    lhsT=mm(v_all[b][kt][:, hh * VW:hh * VW + VW]),
                    rhs=mm(et[:, head * 512:(head + 1) * 512]),
                    start=(kt == 0), stop=(kt == NKT - 1))
            if kt == NKT - 1:
                emit_normalize(i, base)

        def emit_normalize(i, base):
            b, hp, qblk, kt = steps[i]
            ao = ao_cur[(b, hp)]
            qs = slice(qblk * 512, (qblk + 1) * 512)
            # last window gates the whole tail: jump every queue
            last = (b == bl - 1 and hp == NHP - 1)
            stk = tc.high_priority(offset=3000) if last else None
            if stk is not None:
                stk.__enter__()
            for head in (1, 0):
                usb = smp.tile([VW, 512], FP32, tag=f"usb{head}",
                               name=f"usb_b{b}hp{hp}q{qblk}h{head}")
                # gates the U-accumulator bank release: jump the DVE queue
                with tc.high_priority(offset=300):
                    nc.vector.tensor_copy(usb, u_ps.pop((base, head)))
                # Z row -> partition 0 (DMA), broadcast to 64 partitions
                # (gpsimd), then reciprocal on the full-width tile (the
                # custom DVE op mis-executes on 1-partition slices at
                # base partition != 0).
                z1 = smp.tile([1, 512], FP32, tag=f"z1{head}", bufs=1,
                              name=f"z1_b{b}hp{hp}q{qblk}h{head}")
                nc.gpsimd.dma_start(out=z1, in_=usb[hd:hd + 1, :])
                rb = smp.tile([64, 512], FP32, tag=f"rb{head}",
                              name=f"rb_b{b}hp{hp}q{qblk}h{head}")
                nc.gpsimd.partition_broadcast(rb, z1)
                nc.vector.reciprocal_approx_fast(rb, rb)
                if head == 0:
                    nc.vector.tensor_mul(ao[0:64, qs], usb[0:hd, :], rb)
                else:
                    sc = smp.tile([64, 512], SD, tag="sc",
                                  name=f"sc_b{b}hp{hp}q{qblk}")
                    nc.vector.tensor_mul(sc, usb[0:hd, :], rb)
                    nc.gpsimd.dma_start(out=ao[64:128, qs], in_=sc)
            if stk is not None:
                stk.__exit__(None, None, None)
            if qblk == NQB - 1:
                ao_tiles[(b, hp)] = ao

        for i in range(len(steps) + 1):
            if i < len(steps):
                emit_S(i)
            if i > 0:
                pump(1)
                emit_U(i - 1)

        # ---------------- tail: out-proj(b1) ------------------------------
        tail_rot = [("p", ps_p), ("s", ps_s)]
        gi = 0
        for tt in range(NKT):
            for half in range(2):
                ptag, pool = tail_rot[gi % 2]
                emit_out_group(1, tt, half, pool, ptag)
                gi += 1
        # drain any leftover fillers (shouldn't be any)
        while fillers:
            fillers.popleft()()

    nc.compile()
    return nc


_NC_CACHE = {}


def _get_nc(compute=COMPUTE):
    if compute not in _NC_CACHE:
        _NC_CACHE[compute] = build_attention_nc(compute)
    return _NC_CACHE[compute]


def make_in_maps(x, W_qkv, W_proj, b_proj, compute=None):
    compute = compute or COMPUTE
    if compute == "bf16":
        import ml_dtypes
        sd = ml_dtypes.bfloat16
    else:
        sd = np.float32
    x = np.asarray(x, dtype=np.float32)
    W_qkv = np.ascontiguousarray(np.asarray(W_qkv, dtype=np.float32)).astype(sd)
    W_proj = np.ascontiguousarray(np.asarray(W_proj, dtype=np.float32)).astype(sd)
    bias = np.ascontiguousarray(
        np.broadcast_to(np.asarray(b_proj, dtype=np.float32), (128, C)))
    in_maps = []
    for i in range(NCORES):
        shard = x[i * BL:(i + 1) * BL]                      # [BL, N, C]
        xT = np.ascontiguousarray(shard.transpose(2, 0, 1).reshape(C, T)).astype(sd)
        in_maps.append({"xT": xT, "w_qkv": W_qkv, "w_proj": W_proj,
                        "bias": bias})
    return in_maps


def kernel(x, W_qkv, W_proj, b_proj):
    from concourse.bass_utils import run_bass_kernel_spmd

    nc = _get_nc()
    in_maps = make_in_maps(x, W_qkv, W_proj, b_proj)
    res = run_bass_kernel_spmd(nc, in_maps, core_ids=list(range(NCORES)))
    outs = [res.results[i]["out"].reshape(BL, N, C) for i in range(NCORES)]
    return np.concatenate(outs, axis=0).astype(np.float32)


if __name__ == "__main__":
    nc = build_attention_nc()
    print("built ok")


# revision 20
# speedup vs baseline: 1.0099x; 1.0099x over previous
"""Trainium2 Bass kernel: multi-head self-attention block (B=16, N=1024, C=768, H=12).

Data-parallel over batch: 8 NeuronCores x 2 batches each, no collectives.

Dataflow (per core, all-transposed activations; no on-chip transposes):
  host: xT = x_shard^T                                  [C, T]
  qkT  = W_qkv[:, :2C]^T-tiles @ xT   (per batch)       [2C, N]   (q^T | k^T)
  v'   = xT-tiles^T @ W_qkv[:, 2C:]  (+ ones col/head)  [N, H*(HD+1)]
  S^T  = k^T-slices^T @ q^T  (head pair packed in one   [128, 1024]
         2-bank PSUM tile: head A cols 0:512, B 512:)
  E    = exp(SCALE * S^T)     (ONE ScalarE op per step)
  U'   = v'^T @ E  (accum over k; row HD = softmax Z)   [HD+1, 512]
  aoT  = U'[:HD] * (1/Z broadcast)                      [C, N]
  y    = aoT-tiles^T @ W_proj + b                       [N, C]

Scheduling: one global software pipeline over (batch, head-pair, q-block,
k-tile) steps.  S(i+1) is emitted before U(i) so the PE never sits on the
exp latency; all projection work (v-phase, qk projections, out-proj of
batch 0) is chopped into 6-matmul "filler" groups pumped one-per-step into
2 spare PSUM banks, hiding it inside the Scalar-paced attention windows.
PSUM: S-ring 2x[128,1024] (8KB) + U-accum 2x[65,512] (4KB) + filler
2x[128,512] (4KB) = 16KB exactly.
"""

import sys
from collections import deque

for _p in ("/opt/trn_rl_repo", "/opt/pypackages"):
    if _p not in sys.path:
        sys.path.append(_p)

import numpy as np

B, N, C, H = 16, 1024, 768, 12
HD = C // H            # 64
SCALE = HD ** -0.5
NCORES = 8
BL = B // NCORES       # 2 batches per core
T = BL * N             # 2048 tokens per core

COMPUTE = "bf16"       # "bf16" | "f32" | "f32r"


def build_attention_nc(compute=COMPUTE, bl=BL, n=N, c=C, h=H):
    import concourse.bass as bass
    import concourse.tile as tile
    from concourse import bacc, mybir
    from contextlib import ExitStack

    hd = c // h
    t = bl * n
    scale = hd ** -0.5
    assert c % 128 == 0 and n % 512 == 0 and h % 2 == 0 and hd == 64
    CCH = c // 128      # contraction chunks over channels (6)
    NHP = h // 2        # head pairs (6)
    NQB = n // 512      # q-blocks per sequence (2)
    NKT = n // 128      # 128-wide k/token tiles per sequence (8)
    NXH = n // 512      # 512-wide x tiles per sequence (2)
    VW = hd + 1         # v' width per head (ones col at hd)
    PH = c // 2         # proj/v free-dim half (384) <= 1 PSUM bank

    FP32 = mybir.dt.float32
    SD = mybir.dt.bfloat16 if compute == "bf16" else FP32  # storage dtype

    def mm(ap):
        return ap.bitcast(mybir.dt.float32r) if compute == "f32r" else ap

    nc = bacc.Bacc("TRN2", target_bir_lowering=False, debug=False,
                   num_devices=NCORES)

    xT_d = nc.dram_tensor("xT", [c, t], SD, kind="ExternalInput").ap()
    wqkv_d = nc.dram_tensor("w_qkv", [c, 3 * c], SD, kind="ExternalInput").ap()
    wproj_d = nc.dram_tensor("w_proj", [c, c], SD, kind="ExternalInput").ap()
    bias_d = nc.dram_tensor("bias", [128, c], FP32, kind="ExternalInput").ap()
    out_d = nc.dram_tensor("out", [t, c], FP32, kind="ExternalOutput").ap()

    Exp = mybir.ActivationFunctionType.Exp

    with tile.TileContext(nc) as tc, ExitStack() as ctx:
        consts = ctx.enter_context(tc.tile_pool(name="consts", bufs=1))
        xp = ctx.enter_context(tc.tile_pool(name="xp", bufs=2))
        qkp = ctx.enter_context(tc.tile_pool(name="qkp", bufs=2))
        vp = ctx.enter_context(tc.tile_pool(name="vp", bufs=2))
        ep = ctx.enter_context(tc.tile_pool(name="ep", bufs=4))
        aop = ctx.enter_context(tc.tile_pool(name="aop", bufs=2))
        smp = ctx.enter_context(tc.tile_pool(name="smp", bufs=2))
        yp = ctx.enter_context(tc.tile_pool(name="yp", bufs=4))
        # PSUM: 16KB/partition total. s: 2x4KB, u: 2x2KB, p: 2x2KB.
        ps_s = ctx.enter_context(tc.tile_pool(name="ps_s", bufs=2, space="PSUM"))
        ps_u = ctx.enter_context(tc.tile_pool(name="ps_u", bufs=2, space="PSUM"))
        ps_p = ctx.enter_context(tc.tile_pool(name="ps_p", bufs=2, space="PSUM"))

        # ---------------- DMA loads (ordered for the pipeline ramp) -------
        # The load path is one serialized queue at ~320GB/s, so order by
        # first-use: wv + x(b0,xh0) (v-phase), then just the two 128-col
        # slivers of wqk that hp0 needs, then x(b0,xh1), then the rest.
        # Dependency tracking is tile-granular and every DMA transfer costs
        # ~600ns on the serialized load queue, so: hp0's qk weights get their
        # own small merged tiles (no false dep on the bulk wqk load), and the
        # per-cc x halves merge into ONE transfer per (batch, half) via a
        # rearranged access pattern.
        def merged_x(b, xh):
            xt = xp.tile([128, CCH * 512], SD, tag=f"xm{b}_{xh}",
                         name=f"x_b{b}h{xh}")
            src = xT_d[0:c, b * n + xh * 512:b * n + (xh + 1) * 512]
            nc.sync.dma_start(
                out=xt.rearrange("p (cc t) -> p cc t", cc=CCH),
                in_=src.rearrange("(cc p) t -> p cc t", cc=CCH))
            return xt

        wv_sb = []
        wqk_sb = []
        for cc in range(CCH):
            wv = consts.tile([128, c], SD, tag=f"wv{cc}")
            nc.sync.dma_start(out=wv, in_=wqkv_d[cc * 128:(cc + 1) * 128,
                                                 2 * c:3 * c])
            wv_sb.append(wv)
        xm = {(0, 0): merged_x(0, 0)}
        wq0_all = consts.tile([128, CCH * 128], SD, tag="wq0")
        nc.sync.dma_start(
            out=wq0_all.rearrange("p (cc w) -> p cc w", cc=CCH),
            in_=wqkv_d[0:c, 0:128].rearrange("(cc p) w -> p cc w", cc=CCH))
        wk0_all = consts.tile([128, CCH * 128], SD, tag="wk0")
        nc.sync.dma_start(
            out=wk0_all.rearrange("p (cc w) -> p cc w", cc=CCH),
            in_=wqkv_d[0:c, c:c + 128].rearrange("(cc p) w -> p cc w", cc=CCH))
        xm[(0, 1)] = merged_x(0, 1)

        def x_ap(b, cc, xh):
            return xm[(b, xh)][:, cc * 512:(cc + 1) * 512]

        # bulk wqk (used from hp1 on), then x(b1), wproj, bias (mid-program).
        for cc in range(CCH):
            w1 = consts.tile([128, 2 * c], SD, tag=f"wqkv{cc}")
            nc.sync.dma_start(out=w1, in_=wqkv_d[cc * 128:(cc + 1) * 128,
                                                 0:2 * c])
            wqk_sb.append(w1)
        xm[(1, 0)] = merged_x(1, 0)
        xm[(1, 1)] = merged_x(1, 1)
        wproj_sb = []
        for cc in range(CCH):
            w2 = consts.tile([128, c], SD, tag=f"wproj{cc}")
            nc.sync.dma_start(out=w2, in_=wproj_d[cc * 128:(cc + 1) * 128, :])
            wproj_sb.append(w2)
        bias_sb = consts.tile([128, c], FP32, tag="bias")
        nc.sync.dma_start(out=bias_sb, in_=bias_d)

        # ---------------- group emitters ---------------------------------
        # Each emits 6 accumulating matmuls into a PSUM tile from pool/tag,
        # then a high-priority DVE evacuation.

        v_all = [[None] * NKT for _ in range(bl)]

        def v_tile_of(b, tt):
            if v_all[b][tt] is None:
                vt = vp.tile([128, h * VW], SD, tag=f"v{tt}", name=f"v_b{b}t{tt}")
                ones_view = vt[:, :].rearrange("p (hh w) -> p hh w", hh=h)[:, :, hd:hd + 1]
                nc.gpsimd.memset(ones_view, 1.0)
                v_all[b][tt] = vt
            return v_all[b][tt]

        HCC = CCH // 2   # matmuls per filler half-group

        def v_parts(b, tt, half, pool, ptag):
            st = {}

            def part(lo, hi):
                def go():
                    vt = v_tile_of(b, tt)
                    if lo == 0:
                        st["ps"] = pool.tile([128, PH], FP32, tag=ptag,
                                             name=f"vps_b{b}t{tt}f{half}")
                    ps = st["ps"]
                    xh, tl = tt // 4, tt % 4
                    for cc in range(lo, hi):
                        nc.tensor.matmul(
                            ps,
                            lhsT=mm(x_ap(b, cc, xh)[:, tl * 128:(tl + 1) * 128]),
                            rhs=mm(wv_sb[cc][:, half * PH:(half + 1) * PH]),
                            start=(cc == 0), stop=(cc == CCH - 1))
                    if hi == CCH:
                        nheads = PH // hd
                        dst = vt[:, half * nheads * VW:(half + 1) * nheads * VW
                                 ].rearrange("p (hh w) -> p hh w",
                                             hh=nheads)[:, :, 0:hd]
                        srcv = ps[:].rearrange("p (hh w) -> p hh w", hh=nheads)
                        with tc.high_priority(offset=300):
                            nc.vector.tensor_copy(dst, srcv)
                return go
            return [part(0, HCC), part(HCC, CCH)]

        def emit_v_group(b, tt, half, pool, ptag):
            for p in v_parts(b, tt, half, pool, ptag):
                p()

        qk_tiles = {}

        def qk_dst(b, hp, which, qn):
            key = (b, hp, which, qn)
            if key not in qk_tiles:
                qk_tiles[key] = qkp.tile([128, 512], SD, tag=f"{which}{qn}",
                                         name=f"{which}{qn}_b{b}hp{hp}")
            return qk_tiles[key]

        def qk_parts(b, hp, which, qn, pool, ptag):
            st = {}

            def part(lo, hi):
                def go():
                    dst = qk_dst(b, hp, which, qn)
                    if hp == 0:
                        w0 = wq0_all if which == "qt" else wk0_all
                        w_of = lambda cc: w0[:, cc * 128:(cc + 1) * 128]
                    else:
                        fbase = hp * 128 if which == "qt" else c + hp * 128
                        w_of = lambda cc: wqk_sb[cc][:, fbase:fbase + 128]
                    if lo == 0:
                        st["ps"] = pool.tile([128, 512], FP32, tag=ptag,
                                             name=f"qkps_{which}_b{b}hp{hp}q{qn}")
                    ps = st["ps"]
                    for cc in range(lo, hi):
                        nc.tensor.matmul(
                            ps,
                            lhsT=mm(w_of(cc)),
                            rhs=mm(x_ap(b, cc, qn)),
                            start=(cc == 0), stop=(cc == CCH - 1))
                    if hi == CCH:
                        with tc.high_priority(offset=300):
                            nc.vector.tensor_copy(dst, ps)
                return go
            return [part(0, HCC), part(HCC, CCH)]

        def emit_qk_group(b, hp, which, qn, pool, ptag):
            for p in qk_parts(b, hp, which, qn, pool, ptag):
                p()

        ao_tiles = {}

        def out_parts(b, tt, half, pool, ptag):
            st = {}

            def part(lo, hi):
                def go():
                    if lo == 0:
                        st["ps"] = pool.tile([128, PH], FP32, tag=ptag,
                                             name=f"yps_b{b}t{tt}f{half}")
                    ps = st["ps"]
                    for cc in range(lo, hi):
                        nc.tensor.matmul(
                            ps,
                            lhsT=mm(ao_tiles[(b, cc)][:, tt * 128:(tt + 1) * 128]),
                            rhs=mm(wproj_sb[cc][:, half * PH:(half + 1) * PH]),
                            start=(cc == 0), stop=(cc == CCH - 1))
                    if hi == CCH:
                        yt = yp.tile([128, PH], FP32, tag="y",
                                     name=f"y_b{b}t{tt}f{half}")
                        with tc.high_priority(offset=300):
                            nc.vector.tensor_add(
                                yt, ps, bias_sb[:, half * PH:(half + 1) * PH])
                        nc.sync.dma_start(
                            out=out_d[b * n + tt * 128:b * n + (tt + 1) * 128,
                                      half * PH:(half + 1) * PH],
                            in_=yt)
                return go
            return [part(0, HCC), part(HCC, CCH)]

        def emit_out_group(b, tt, half, pool, ptag):
            for p in out_parts(b, tt, half, pool, ptag):
                p()

        # ---------------- filler pump ------------------------------------
        fillers = deque()

        def pump(k=1):
            for _ in range(k):
                if fillers:
                    fillers.popleft()()

        # ---------------- head phase -------------------------------------
        # Just enough to start attention: v(b0, tt0) both halves + qk(b0,0).
        # Rotate over the (still idle) u and p slots for double buffering.
        head_rot = [("u", ps_u), ("p", ps_p)]
        head_groups = [lambda pool, ptag: emit_v_group(0, 0, 0, pool, ptag),
                       lambda pool, ptag: emit_v_group(0, 0, 1, pool, ptag)]
        for qn in range(NQB):
            for which in ("qt", "kt"):
                head_groups.append(
                    (lambda which=which, qn=qn: lambda pool, ptag:
                     emit_qk_group(0, 0, which, qn, pool, ptag))())
        for i, g in enumerate(head_groups):
            ptag, pool = head_rot[i % 2]
            g(pool, ptag)

        # ---------------- window filler schedule --------------------------
        # Each entry is one pump event: either a whole v group (JIT, hp0) or
        # a 3-matmul half-group. 16 pump slots per window (one per kt-step).
        # v halves: half 0 feeds head pairs 0..2, half 1 feeds 3..5.
        def qk_f(b, hp):
            out = []
            for qn in range(NQB):
                for which in ("qt", "kt"):
                    out.extend(qk_parts(b, hp, which, qn, ps_p, "p"))
            return out

        def v_f(b, tts, half):
            # whole groups: one LDW stall amortized over 6 matmuls
            return [(lambda tt=tt: emit_v_group(b, tt, half, ps_p, "p"))
                    for tt in tts]

        def window_fillers(b, hp):
            if b == 0:
                if hp == 0:
                    # JIT v(b0) half-0 tiles tt1..7 as whole groups: tt_k at
                    # pump slot k-1 lands just before U(kt=k-1).
                    return [(lambda tt=tt: emit_v_group(0, tt, 0, ps_p, "p"))
                            for tt in range(1, NKT)] + qk_f(0, 1)
                if hp == 1:
                    return qk_f(0, 2) + v_f(0, range(0, 4), 1)
                if hp == 2:
                    return qk_f(0, 3) + v_f(0, range(4, NKT), 1)
                if hp == 3:
                    return qk_f(0, 4) + v_f(1, range(0, 4), 0)
                if hp == 4:
                    return qk_f(0, 5) + v_f(1, range(4, NKT), 0)
                return qk_f(1, 0)
            else:
                all_out = [(tt, half) for tt in range(NKT) for half in range(2)]
                out = []
                if hp < NHP - 1:
                    out += qk_f(1, hp + 1)
                if hp == 0:
                    out += v_f(1, range(0, 4), 1)
                elif hp == 1:
                    out += v_f(1, range(4, NKT), 1)
                else:
                    for tt, half in all_out[(hp - 2) * 4:(hp - 1) * 4]:
                        out.append(lambda tt=tt, half=half:
                                   emit_out_group(0, tt, half, ps_p, "p"))
                return out

        # ---------------- attention: global software pipeline -------------
        steps = [(b, hp, qblk, kt)
                 for b in range(bl)
                 for hp in range(NHP)
                 for qblk in range(NQB)
                 for kt in range(NKT)]

        u_ps = {}     # (qblk % 2, head) -> psum accum tile (ring by alloc)
        et_by_step = {}
        ao_cur = {}   # (b, hp) -> ao tile

        def emit_S(i):
            b, hp, qblk, kt = steps[i]
            if qblk == 0 and kt == 0:
                ao_cur[(b, hp)] = aop.tile([128, n], SD, tag=f"ao{hp}",
                                           name=f"ao_b{b}hp{hp}")
                fillers.extend(window_fillers(b, hp))
            if kt == 0:
                for head in range(2):
                    u_ps[(i, head)] = ps_u.tile(
                        [VW, 512], FP32, tag="u",
                        name=f"u_b{b}hp{hp}q{qblk}h{head}")
            qt_t = qk_dst(b, hp, "qt", qblk)
            kt_t = qk_dst(b, hp, "kt", kt // 4)
            ko = (kt % 4) * 128
            sps = ps_s.tile([128, 2 * 512], FP32, tag="s",
                            name=f"s_b{b}hp{hp}q{qblk}k{kt}")
            for head in range(2):
                p0 = head * 64
                nc.tensor.matmul(
                    sps[:, head * 512:(head + 1) * 512],
                    lhsT=mm(kt_t[p0:p0 + 64, ko:ko + 128]),
                    rhs=mm(qt_t[p0:p0 + 64, :]),
                    start=True, stop=True)
            et = ep.tile([128, 2 * 512], SD, tag="e",
                         name=f"e_b{b}hp{hp}q{qblk}k{kt}")
            nc.scalar.activation(et, sps, Exp, scale=scale)
            et_by_step[i] = et

        def emit_U(i):
            b, hp, qblk, kt = steps[i]
            et = et_by_step.pop(i)
            base = i - kt
            for head in range(2):
                hh = 2 * hp + head
                nc.tensor.matmul(
                    u_ps[(base, head)],
                    lhsT=mm(v_all[b][kt][:, hh * VW:hh * VW + VW]),
                    rhs=mm(et[:, head * 512:(head + 1) * 512]),
                    start=(kt == 0), stop=(kt == NKT - 1))
            if kt == NKT - 1:
                emit_normalize(i, base)

        def emit_normalize(i, base):
            b, hp, qblk, kt = steps[i]
            ao = ao_cur[(b, hp)]
            qs = slice(qblk * 512, (qblk + 1) * 512)
            # last window gates the whole tail: jump every queue
            last = (b == bl - 1 and hp == NHP - 1)
            stk = tc.high_priority(offset=3000) if last else None
            if stk is not None:
                stk.__enter__()
            for head in (1, 0):
                usb = smp.tile([VW, 512], FP32, tag=f"usb{head}",
                               name=f"usb_b{b}hp{hp}q{qblk}h{head}")
                # gates the U-accumulator bank release: jump the DVE queue
                with tc.high_priority(offset=300):
                    nc.vector.tensor_copy(usb, u_ps.pop((base, head)))
                # Z row -> partition 0 (DMA), broadcast to 64 partitions
                # (gpsimd), then reciprocal on the full-width tile (the
                # custom DVE op mis-executes on 1-partition slices at
                # base partition != 0).
                z1 = smp.tile([1, 512], FP32, tag=f"z1{head}", bufs=1,
                              name=f"z1_b{b}hp{hp}q{qblk}h{head}")
                nc.gpsimd.dma_start(out=z1, in_=usb[hd:hd + 1, :])
                rb = smp.tile([64, 512], FP32, tag=f"rb{head}",
                              name=f"rb_b{b}hp{hp}q{qblk}h{head}")
                nc.gpsimd.partition_broadcast(rb, z1)
                nc.vector.reciprocal_approx_fast(rb, rb)
                if head == 0:
                    nc.vector.tensor_mul(ao[0:64, qs], usb[0:hd, :], rb)
                else:
                    sc = smp.tile([64, 512], SD, tag="sc",
                                  name=f"sc_b{b}hp{hp}q{qblk}")
                    nc.vector.tensor_mul(sc, usb[0:hd, :], rb)
                    nc.gpsimd.dma_start(out=ao[64:128, qs], in_=sc)
            if stk is not None:
                stk.__exit__(None, None, None)
            if qblk == NQB - 1:
                ao_tiles[(b, hp)] = ao

        for i in range(len(steps) + 1):
            if i < len(steps):
                emit_S(i)
            if i > 0:
                # drain the v-JIT + qk backlog fast in the first window so
                # hp1's q/k tiles are ready when its first S issues
                pump(2 if steps[i - 1][:2] == (0, 0) else 1)
                emit_U(i - 1)

        # ---------------- tail: out-proj(b1) ------------------------------
        tail_rot = [("p", ps_p), ("s", ps_s)]
        gi = 0
        for tt in range(NKT):
            for half in range(2):
                ptag, pool = tail_rot[gi % 2]
                emit_out_group(1, tt, half, pool, ptag)
                gi += 1
        # drain any leftover fillers (shouldn't be any)
        while fillers:
            fillers.popleft()()

    nc.compile()
    return nc


_NC_CACHE = {}


def _get_nc(compute=COMPUTE):
    if compute not in _NC_CACHE:
        _NC_CACHE[compute] = build_attention_nc(compute)
    return _NC_CACHE[compute]


def make_in_maps(x, W_qkv, W_proj, b_proj, compute=None):
    compute = compute or COMPUTE
    if compute == "bf16":
        import ml_dtypes
        sd = ml_dtypes.bfloat16
    else:
        sd = np.float32
    x = np.asarray(x, dtype=np.float32)
    W_qkv = np.ascontiguousarray(np.asarray(W_qkv, dtype=np.float32)).astype(sd)
    W_proj = np.ascontiguousarray(np.asarray(W_proj, dtype=np.float32)).astype(sd)
    bias = np.ascontiguousarray(
        np.broadcast_to(np.asarray(b_proj, dtype=np.float32), (128, C)))
    in_maps = []
    for i in range(NCORES):
        shard = x[i * BL:(i + 1) * BL]                      # [BL, N, C]
        xT = np.ascontiguousarray(shard.transpose(2, 0, 1).reshape(C, T)).astype(sd)
        in_maps.append({"xT": xT, "w_qkv": W_qkv, "w_proj": W_proj,
                        "bias": bias})
    return in_maps


def kernel(x, W_qkv, W_proj, b_proj):
    from concourse.bass_utils import run_bass_kernel_spmd

    nc = _get_nc()
    in_maps = make_in_maps(x, W_qkv, W_proj, b_proj)
    res = run_bass_kernel_spmd(nc, in_maps, core_ids=list(range(NCORES)))
    outs = [res.results[i]["out"].reshape(BL, N, C) for i in range(NCORES)]
    return np.concatenate(outs, axis=0).astype(np.float32)


if __name__ == "__main__":
    nc = build_attention_nc()
    print("built ok")


# revision 28
# speedup vs baseline: 1.0274x; 1.0173x over previous
"""Trainium2 Bass kernel: multi-head self-attention block (B=16, N=1024, C=768, H=12).

Data-parallel over batch: 8 NeuronCores x 2 batches each, no collectives.

Dataflow (per core, all-transposed activations; no on-chip transposes):
  host: xT = x_shard^T                                  [C, T]
  qkT  = W_qkv[:, :2C]^T-tiles @ xT   (per batch)       [2C, N]   (q^T | k^T)
  v'   = xT-tiles^T @ W_qkv[:, 2C:]  (+ ones col/head)  [N, H*(HD+1)]
  S^T  = k^T-slices^T @ q^T  (head pair packed in one   [128, 1024]
         2-bank PSUM tile: head A cols 0:512, B 512:)
  E    = exp(SCALE * S^T)     (ONE ScalarE op per step)
  U'   = v'^T @ E  (accum over k; row HD = softmax Z)   [HD+1, 512]
  aoT  = U'[:HD] * (1/Z broadcast)                      [C, N]
  y    = aoT-tiles^T @ W_proj + b                       [N, C]

Scheduling: one global software pipeline over (batch, head-pair, q-block,
k-tile) steps.  S(i+1) is emitted before U(i) so the PE never sits on the
exp latency; all projection work (v-phase, qk projections, out-proj of
batch 0) is chopped into 6-matmul "filler" groups pumped one-per-step into
2 spare PSUM banks, hiding it inside the Scalar-paced attention windows.
PSUM: S-ring 2x[128,1024] (8KB) + U-accum 2x[65,512] (4KB) + filler
2x[128,512] (4KB) = 16KB exactly.
"""

import sys
from collections import deque

for _p in ("/opt/trn_rl_repo", "/opt/pypackages"):
    if _p not in sys.path:
        sys.path.append(_p)

import numpy as np

B, N, C, H = 16, 1024, 768, 12
HD = C // H            # 64
SCALE = HD ** -0.5
NCORES = 8
BL = B // NCORES       # 2 batches per core
T = BL * N             # 2048 tokens per core

COMPUTE = "bf16"       # "bf16" | "f32" | "f32r"


def build_attention_nc(compute=COMPUTE, bl=BL, n=N, c=C, h=H):
    import concourse.bass as bass
    import concourse.tile as tile
    from concourse import bacc, mybir
    from contextlib import ExitStack

    hd = c // h
    t = bl * n
    scale = hd ** -0.5
    assert c % 128 == 0 and n % 512 == 0 and h % 2 == 0 and hd == 64
    CCH = c // 128      # contraction chunks over channels (6)
    NHP = h // 2        # head pairs (6)
    NQB = n // 512      # q-blocks per sequence (2)
    NKT = n // 128      # 128-wide k/token tiles per sequence (8)
    NXH = n // 512      # 512-wide x tiles per sequence (2)
    VW = hd + 1         # v' width per head (ones col at hd)
    PH = c // 2         # proj/v free-dim half (384) <= 1 PSUM bank

    FP32 = mybir.dt.float32
    SD = mybir.dt.bfloat16 if compute == "bf16" else FP32  # storage dtype

    def mm(ap):
        return ap.bitcast(mybir.dt.float32r) if compute == "f32r" else ap

    nc = bacc.Bacc("TRN2", target_bir_lowering=False, debug=False,
                   num_devices=NCORES)

    xT_d = nc.dram_tensor("xT", [c, t], SD, kind="ExternalInput").ap()
    wqkv_d = nc.dram_tensor("w_qkv", [c, 3 * c], SD, kind="ExternalInput").ap()
    wproj_d = nc.dram_tensor("w_proj", [c, c], SD, kind="ExternalInput").ap()
    bias_d = nc.dram_tensor("bias", [128, c], FP32, kind="ExternalInput").ap()
    out_d = nc.dram_tensor("out", [t, c], FP32, kind="ExternalOutput").ap()

    Exp = mybir.ActivationFunctionType.Exp

    with tile.TileContext(nc) as tc, ExitStack() as ctx:
        consts = ctx.enter_context(tc.tile_pool(name="consts", bufs=1))
        xp = ctx.enter_context(tc.tile_pool(name="xp", bufs=2))
        qkp = ctx.enter_context(tc.tile_pool(name="qkp", bufs=2))
        vp = ctx.enter_context(tc.tile_pool(name="vp", bufs=2))
        ep = ctx.enter_context(tc.tile_pool(name="ep", bufs=4))
        aop = ctx.enter_context(tc.tile_pool(name="aop", bufs=2))
        smp = ctx.enter_context(tc.tile_pool(name="smp", bufs=2))
        yp = ctx.enter_context(tc.tile_pool(name="yp", bufs=4))
        # PSUM: 16KB/partition total. s: 2x4KB, u: 2x2KB, p: 2x2KB.
        ps_s = ctx.enter_context(tc.tile_pool(name="ps_s", bufs=2, space="PSUM"))
        ps_u = ctx.enter_context(tc.tile_pool(name="ps_u", bufs=2, space="PSUM"))
        ps_p = ctx.enter_context(tc.tile_pool(name="ps_p", bufs=2, space="PSUM"))

        # ---------------- DMA loads (ordered for the pipeline ramp) -------
        # The load path is one serialized queue at ~320GB/s, so order by
        # first-use: wv + x(b0,xh0) (v-phase), then just the two 128-col
        # slivers of wqk that hp0 needs, then x(b0,xh1), then the rest.
        # Dependency tracking is tile-granular and every DMA transfer costs
        # ~600ns on the serialized load queue, so: hp0's qk weights get their
        # own small merged tiles (no false dep on the bulk wqk load), and the
        # per-cc x halves merge into ONE transfer per (batch, half) via a
        # rearranged access pattern.
        # Three parallel load queues (Sync, Scalar, GpSimd): weights on Sync,
        # x(b0) on Scalar (idle until the first exp), hp0's qk slivers on
        # GpSimd. Each queue serializes at ~320GB/s with ~600ns/transfer
        # minimum, so the head-critical bytes split across all three.
        def merged_x(b, xh, half, eng):
            xt = xp.tile([128, 3 * 512], SD, tag=f"xm{b}_{xh}_{half}",
                         name=f"x_b{b}h{xh}q{half}")
            c0 = half * 3 * 128
            src = xT_d[c0:c0 + 3 * 128,
                       b * n + xh * 512:b * n + (xh + 1) * 512]
            eng.dma_start(
                out=xt.rearrange("p (cc t) -> p cc t", cc=3),
                in_=src.rearrange("(cc p) t -> p cc t", cc=3))
            return xt

        # PE warmup: ~8 throwaway matmuls ramp the PE clock to full speed
        # before the first real group issues.
        wrm = consts.tile([128, 512], SD, tag="warm")
        nc.gpsimd.memset(wrm, 0.0)
        xm = {}
        wq0_all = consts.tile([128, CCH * 128], SD, tag="wq0")
        nc.gpsimd.dma_start(
            out=wq0_all.rearrange("p (cc w) -> p cc w", cc=CCH),
            in_=wqkv_d[0:c, 0:128].rearrange("(cc p) w -> p cc w", cc=CCH))
        wk0_all = consts.tile([128, CCH * 128], SD, tag="wk0")
        nc.gpsimd.dma_start(
            out=wk0_all.rearrange("p (cc w) -> p cc w", cc=CCH),
            in_=wqkv_d[0:c, c:c + 128].rearrange("(cc p) w -> p cc w", cc=CCH))
        for xh in range(NXH):
            for half in range(2):
                xm[(0, xh, half)] = merged_x(0, xh, half, nc.scalar)
        wv_sb = []
        wqk_sb = []
        for cc in range(CCH):
            wv = consts.tile([128, c], SD, tag=f"wv{cc}")
            nc.sync.dma_start(out=wv, in_=wqkv_d[cc * 128:(cc + 1) * 128,
                                                 2 * c:3 * c])
            wv_sb.append(wv)

        def x_ap(b, cc, xh):
            return xm[(b, xh, cc // 3)][:, (cc % 3) * 512:(cc % 3 + 1) * 512]

        for _ in range(8):
            wps = ps_s.tile([128, 512], FP32, tag="s", name="warmup")
            nc.tensor.matmul(wps, lhsT=mm(wrm[:, 0:128]), rhs=mm(wrm),
                             start=True, stop=True)

        # bulk wqk (used from hp1 on), then x(b1), wproj, bias (mid-program).
        for cc in range(CCH):
            w1 = consts.tile([128, 2 * c], SD, tag=f"wqkv{cc}")
            nc.sync.dma_start(out=w1, in_=wqkv_d[cc * 128:(cc + 1) * 128,
                                                 0:2 * c])
            wqk_sb.append(w1)
        for xh in range(NXH):
            for half in range(2):
                xm[(1, xh, half)] = merged_x(1, xh, half, nc.sync)
        wproj_sb = []
        for cc in range(CCH):
            w2 = consts.tile([128, c], SD, tag=f"wproj{cc}")
            nc.sync.dma_start(out=w2, in_=wproj_d[cc * 128:(cc + 1) * 128, :])
            wproj_sb.append(w2)
        bias_sb = consts.tile([128, c], FP32, tag="bias")
        nc.sync.dma_start(out=bias_sb, in_=bias_d)

        # ---------------- group emitters ---------------------------------
        # Each emits 6 accumulating matmuls into a PSUM tile from pool/tag,
        # then a high-priority DVE evacuation.

        v_all = [[None] * NKT for _ in range(bl)]

        def v_tile_of(b, tt):
            if v_all[b][tt] is None:
                vt = vp.tile([128, h * VW], SD, tag=f"v{tt}", name=f"v_b{b}t{tt}")
                ones_view = vt[:, :].rearrange("p (hh w) -> p hh w", hh=h)[:, :, hd:hd + 1]
                nc.gpsimd.memset(ones_view, 1.0)
                v_all[b][tt] = vt
            return v_all[b][tt]

        HCC = CCH // 2   # matmuls per filler half-group

        def v_parts(b, tt, half, pool, ptag):
            st = {}

            def part(lo, hi):
                def go():
                    vt = v_tile_of(b, tt)
                    if lo == 0:
                        st["ps"] = pool.tile([128, PH], FP32, tag=ptag,
                                             name=f"vps_b{b}t{tt}f{half}")
                    ps = st["ps"]
                    xh, tl = tt // 4, tt % 4
                    for cc in range(lo, hi):
                        nc.tensor.matmul(
                            ps,
                            lhsT=mm(x_ap(b, cc, xh)[:, tl * 128:(tl + 1) * 128]),
                            rhs=mm(wv_sb[cc][:, half * PH:(half + 1) * PH]),
                            start=(cc == 0), stop=(cc == CCH - 1))
                    if hi == CCH:
                        nheads = PH // hd
                        dst = vt[:, half * nheads * VW:(half + 1) * nheads * VW
                                 ].rearrange("p (hh w) -> p hh w",
                                             hh=nheads)[:, :, 0:hd]
                        srcv = ps[:].rearrange("p (hh w) -> p hh w", hh=nheads)
                        with tc.high_priority(offset=300):
                            nc.vector.tensor_copy(dst, srcv)
                return go
            return [part(0, HCC), part(HCC, CCH)]

        def emit_v_group(b, tt, half, pool, ptag):
            for p in v_parts(b, tt, half, pool, ptag):
                p()

        qk_tiles = {}

        def qk_dst(b, hp, which, qn):
            key = (b, hp, which, qn)
            if key not in qk_tiles:
                qk_tiles[key] = qkp.tile([128, 512], SD, tag=f"{which}{qn}",
                                         name=f"{which}{qn}_b{b}hp{hp}")
            return qk_tiles[key]

        def qk_parts(b, hp, which, qn, pool, ptag):
            st = {}

            def part(lo, hi):
                def go():
                    dst = qk_dst(b, hp, which, qn)
                    if hp == 0:
                        w0 = wq0_all if which == "qt" else wk0_all
                        w_of = lambda cc: w0[:, cc * 128:(cc + 1) * 128]
                    else:
                        fbase = hp * 128 if which == "qt" else c + hp * 128
                        w_of = lambda cc: wqk_sb[cc][:, fbase:fbase + 128]
                    if lo == 0:
                        st["ps"] = pool.tile([128, 512], FP32, tag=ptag,
                                             name=f"qkps_{which}_b{b}hp{hp}q{qn}")
                    ps = st["ps"]
                    for cc in range(lo, hi):
                        nc.tensor.matmul(
                            ps,
                            lhsT=mm(w_of(cc)),
                            rhs=mm(x_ap(b, cc, qn)),
                            start=(cc == 0), stop=(cc == CCH - 1))
                    if hi == CCH:
                        with tc.high_priority(offset=300):
                            nc.vector.tensor_copy(dst, ps)
                return go
            return [part(0, HCC), part(HCC, CCH)]

        def emit_qk_group(b, hp, which, qn, pool, ptag):
            for p in qk_parts(b, hp, which, qn, pool, ptag):
                p()

        ao_tiles = {}

        def out_parts(b, tt, half, pool, ptag):
            st = {}

            def part(lo, hi):
                def go():
                    if lo == 0:
                        st["ps"] = pool.tile([128, PH], FP32, tag=ptag,
                                             name=f"yps_b{b}t{tt}f{half}")
                    ps = st["ps"]
                    for cc in range(lo, hi):
                        nc.tensor.matmul(
                            ps,
                            lhsT=mm(ao_tiles[(b, cc)][:, tt * 128:(tt + 1) * 128]),
                            rhs=mm(wproj_sb[cc][:, half * PH:(half + 1) * PH]),
                            start=(cc == 0), stop=(cc == CCH - 1))
                    if hi == CCH:
                        yt = yp.tile([128, PH], FP32, tag="y",
                                     name=f"y_b{b}t{tt}f{half}")
                        with tc.high_priority(offset=300):
                            nc.vector.tensor_add(
                                yt, ps, bias_sb[:, half * PH:(half + 1) * PH])
                        nc.sync.dma_start(
                            out=out_d[b * n + tt * 128:b * n + (tt + 1) * 128,
                                      half * PH:(half + 1) * PH],
                            in_=yt)
                return go
            return [part(0, HCC), part(HCC, CCH)]

        def emit_out_group(b, tt, half, pool, ptag):
            for p in out_parts(b, tt, half, pool, ptag):
                p()

        # ---------------- filler pump ------------------------------------
        fillers = deque()

        def pump(k=1):
            for _ in range(k):
                if fillers:
                    fillers.popleft()()

        # ---------------- head phase -------------------------------------
        # Just enough to start attention: v(b0, tt0) both halves + qk(b0,0).
        # Rotate over the (still idle) u and p slots for double buffering.
        head_rot = [("u", ps_u), ("p", ps_p)]
        head_groups = []
        for qn in range(NQB):
            for which in ("qt", "kt"):
                head_groups.append(
                    (lambda which=which, qn=qn: lambda pool, ptag:
                     emit_qk_group(0, 0, which, qn, pool, ptag))())
        for i, g in enumerate(head_groups):
            ptag, pool = head_rot[i % 2]
            g(pool, ptag)

        # ---------------- window filler schedule --------------------------
        # Each entry is one pump event: either a whole v group (JIT, hp0) or
        # a 3-matmul half-group. 16 pump slots per window (one per kt-step).
        # v halves: half 0 feeds head pairs 0..2, half 1 feeds 3..5.
        def qk_f(b, hp):
            out = []
            for qn in range(NQB):
                for which in ("qt", "kt"):
                    out.extend(qk_parts(b, hp, which, qn, ps_p, "p"))
            return out

        def v_f(b, tts, half):
            # whole groups: one LDW stall amortized over 6 matmuls
            return [(lambda tt=tt: emit_v_group(b, tt, half, ps_p, "p"))
                    for tt in tts]

        def window_fillers(b, hp):
            if b == 0:
                if hp == 0:
                    # JIT v(b0) half-0 as whole groups (tt0 both halves
                    # first); with pump rate 2 early, v tt_k lands just
                    # before U(kt=k) needs it.
                    return [lambda: emit_v_group(0, 0, 0, ps_p, "p"),
                            lambda: emit_v_group(0, 0, 1, ps_p, "p")] + \
                           [(lambda tt=tt: emit_v_group(0, tt, 0, ps_p, "p"))
                            for tt in range(1, NKT)] + qk_f(0, 1)
                if hp == 1:
                    return qk_f(0, 2) + v_f(0, range(0, 4), 1)
                if hp == 2:
                    return qk_f(0, 3) + v_f(0, range(4, NKT), 1)
                if hp == 3:
                    return qk_f(0, 4) + v_f(1, range(0, 4), 0)
                if hp == 4:
                    return qk_f(0, 5) + v_f(1, range(4, NKT), 0)
                return qk_f(1, 0)
            else:
                all_out = [(tt, half) for tt in range(NKT) for half in range(2)]
                out = []
                if hp < NHP - 1:
                    out += qk_f(1, hp + 1)
                if hp == 0:
                    out += v_f(1, range(0, 4), 1)
                elif hp == 1:
                    out += v_f(1, range(4, NKT), 1)
                else:
                    for tt, half in all_out[(hp - 2) * 4:(hp - 1) * 4]:
                        out.append(lambda tt=tt, half=half:
                                   emit_out_group(0, tt, half, ps_p, "p"))
                return out

        # ---------------- attention: global software pipeline -------------
        steps = [(b, hp, qblk, kt)
                 for b in range(bl)
                 for hp in range(NHP)
                 for qblk in range(NQB)
                 for kt in range(NKT)]

        u_ps = {}     # (qblk % 2, head) -> psum accum tile (ring by alloc)
        et_by_step = {}
        ao_cur = {}   # (b, hp) -> ao tile

        def emit_S(i):
            b, hp, qblk, kt = steps[i]
            if qblk == 0 and kt == 0:
                ao_cur[(b, hp)] = aop.tile([128, n], SD, tag=f"ao{hp}",
                                           name=f"ao_b{b}hp{hp}")
                fillers.extend(window_fillers(b, hp))
            qt_t = qk_dst(b, hp, "qt", qblk)
            kt_t = qk_dst(b, hp, "kt", kt // 4)
            ko = (kt % 4) * 128
            sps = ps_s.tile([128, 2 * 512], FP32, tag="s",
                            name=f"s_b{b}hp{hp}q{qblk}k{kt}")
            for head in range(2):
                p0 = head * 64
                nc.tensor.matmul(
                    sps[:, head * 512:(head + 1) * 512],
                    lhsT=mm(kt_t[p0:p0 + 64, ko:ko + 128]),
                    rhs=mm(qt_t[p0:p0 + 64, :]),
                    start=True, stop=True)
            et = ep.tile([128, 2 * 512], SD, tag="e",
                         name=f"e_b{b}hp{hp}q{qblk}k{kt}")
            nc.scalar.activation(et, sps, Exp, scale=scale)
            et_by_step[i] = et

        def emit_U(i):
            b, hp, qblk, kt = steps[i]
            et = et_by_step.pop(i)
            base = i - kt
            if kt == 0:
                for head in range(2):
                    u_ps[(base, head)] = ps_u.tile(
                        [VW, 512], FP32, tag="u",
                        name=f"u_b{b}hp{hp}q{qblk}h{head}")
            for head in range(2):
                hh = 2 * hp + head
                nc.tensor.matmul(
                    u_ps[(base, head)],
                    lhsT=mm(v_all[b][kt][:, hh * VW:hh * VW + VW]),
                    rhs=mm(et[:, head * 512:(head + 1) * 512]),
                    start=(kt == 0), stop=(kt == NKT - 1))
            if kt == NKT - 1:
                emit_normalize(i, base)

        def emit_normalize(i, base):
            b, hp, qblk, kt = steps[i]
            ao = ao_cur[(b, hp)]
            qs = slice(qblk * 512, (qblk + 1) * 512)
            # the whole chain runs at high priority so next-window fillers
            # can't starve it on DVE/GpSimd (its copies gate U-bank release);
            # the last window gates the whole tail: jump even further
            last = (b == bl - 1 and hp == NHP - 1)
            stk = tc.high_priority(offset=3000 if last else 300)
            stk.__enter__()
            for head in (1, 0):
                usb = smp.tile([VW, 512], FP32, tag=f"usb{head}",
                               name=f"usb_b{b}hp{hp}q{qblk}h{head}")
                # gates the U-accumulator bank release: jump the DVE queue
                with tc.high_priority(offset=300):
                    nc.vector.tensor_copy(usb, u_ps.pop((base, head)))
                # Z row -> partition 0 (DMA), broadcast to 64 partitions
                # (gpsimd), then reciprocal on the full-width tile (the
                # custom DVE op mis-executes on 1-partition slices at
                # base partition != 0).
                z1 = smp.tile([1, 512], FP32, tag=f"z1{head}", bufs=1,
                              name=f"z1_b{b}hp{hp}q{qblk}h{head}")
                nc.gpsimd.dma_start(out=z1, in_=usb[hd:hd + 1, :])
                rb = smp.tile([64, 512], FP32, tag=f"rb{head}",
                              name=f"rb_b{b}hp{hp}q{qblk}h{head}")
                nc.gpsimd.partition_broadcast(rb, z1)
                nc.vector.reciprocal_approx_fast(rb, rb)
                if head == 0:
                    nc.vector.tensor_mul(ao[0:64, qs], usb[0:hd, :], rb)
                else:
                    sc = smp.tile([64, 512], SD, tag="sc",
                                  name=f"sc_b{b}hp{hp}q{qblk}")
                    nc.vector.tensor_mul(sc, usb[0:hd, :], rb)
                    nc.gpsimd.dma_start(out=ao[64:128, qs], in_=sc)
            stk.__exit__(None, None, None)
            if qblk == NQB - 1:
                ao_tiles[(b, hp)] = ao

        for i in range(len(steps) + 1):
            if i < len(steps):
                emit_S(i)
            if i > 0:
                # drain the v-JIT backlog fast at the start of the first
                # window (v tt_k must land before U(kt=k) issues)
                pump(2 if (steps[i - 1][:2] == (0, 0) and i <= 6) else 1)
                emit_U(i - 1)

        # ---------------- tail: out-proj(b1) ------------------------------
        tail_rot = [("p", ps_p), ("s", ps_s)]
        gi = 0
        for tt in range(NKT):
            for half in range(2):
                ptag, pool = tail_rot[gi % 2]
                emit_out_group(1, tt, half, pool, ptag)
                gi += 1
        # drain any leftover fillers (shouldn't be any)
        while fillers:
            fillers.popleft()()

    nc.compile()
    return nc


_NC_CACHE = {}


def _get_nc(compute=COMPUTE):
    if compute not in _NC_CACHE:
        _NC_CACHE[compute] = build_attention_nc(compute)
    return _NC_CACHE[compute]


def make_in_maps(x, W_qkv, W_proj, b_proj, compute=None):
    compute = compute or COMPUTE
    if compute == "bf16":
        import ml_dtypes
        sd = ml_dtypes.bfloat16
    else:
        sd = np.float32
    x = np.asarray(x, dtype=np.float32)
    W_qkv = np.ascontiguousarray(np.asarray(W_qkv, dtype=np.float32)).astype(sd)
    W_proj = np.ascontiguousarray(np.asarray(W_proj, dtype=np.float32)).astype(sd)
    bias = np.ascontiguousarray(
        np.broadcast_to(np.asarray(b_proj, dtype=np.float32), (128, C)))
    in_maps = []
    for i in range(NCORES):
        shard = x[i * BL:(i + 1) * BL]                      # [BL, N, C]
        xT = np.ascontiguousarray(shard.transpose(2, 0, 1).reshape(C, T)).astype(sd)
        in_maps.append({"xT": xT, "w_qkv": W_qkv, "w_proj": W_proj,
                        "bias": bias})
    return in_maps


def kernel(x, W_qkv, W_proj, b_proj):
    from concourse.bass_utils import run_bass_kernel_spmd

    nc = _get_nc()
    in_maps = make_in_maps(x, W_qkv, W_proj, b_proj)
    res = run_bass_kernel_spmd(nc, in_maps, core_ids=list(range(NCORES)))
    outs = [res.results[i]["out"].reshape(BL, N, C) for i in range(NCORES)]
    return np.concatenate(outs, axis=0).astype(np.float32)


if __name__ == "__main__":
    nc = build_attention_nc()
    print("built ok")


# revision 33
# speedup vs baseline: 1.0308x; 1.0033x over previous
"""Trainium2 Bass kernel: multi-head self-attention block (B=16, N=1024, C=768, H=12).

Data-parallel over batch: 8 NeuronCores x 2 batches each, no collectives.

Dataflow (per core, all-transposed activations; no on-chip transposes):
  host: xT = x_shard^T                                  [C, T]
  qkT  = W_qkv[:, :2C]^T-tiles @ xT   (per batch)       [2C, N]   (q^T | k^T)
  v'   = xT-tiles^T @ W_qkv[:, 2C:]  (+ ones col/head)  [N, H*(HD+1)]
  S^T  = k^T-slices^T @ q^T  (head pair packed in one   [128, 1024]
         2-bank PSUM tile: head A cols 0:512, B 512:)
  E    = exp(SCALE * S^T)     (ONE ScalarE op per step)
  U'   = v'^T @ E  (accum over k; row HD = softmax Z)   [HD+1, 512]
  aoT  = U'[:HD] * (1/Z broadcast)                      [C, N]
  y    = aoT-tiles^T @ W_proj + b                       [N, C]

Scheduling: one global software pipeline over (batch, head-pair, q-block,
k-tile) steps.  S(i+1) is emitted before U(i) so the PE never sits on the
exp latency; all projection work (v-phase, qk projections, out-proj of
batch 0) is chopped into 6-matmul "filler" groups pumped one-per-step into
2 spare PSUM banks, hiding it inside the Scalar-paced attention windows.
PSUM: S-ring 2x[128,1024] (8KB) + U-accum 2x[65,512] (4KB) + filler
2x[128,512] (4KB) = 16KB exactly.
"""

import sys
from collections import deque

for _p in ("/opt/trn_rl_repo", "/opt/pypackages"):
    if _p not in sys.path:
        sys.path.append(_p)

import numpy as np

B, N, C, H = 16, 1024, 768, 12
HD = C // H            # 64
SCALE = HD ** -0.5
NCORES = 8
BL = B // NCORES       # 2 batches per core
T = BL * N             # 2048 tokens per core

COMPUTE = "bf16"       # "bf16" | "f32" | "f32r"


def build_attention_nc(compute=COMPUTE, bl=BL, n=N, c=C, h=H):
    import concourse.bass as bass
    import concourse.tile as tile
    from concourse import bacc, mybir
    from contextlib import ExitStack

    hd = c // h
    t = bl * n
    scale = hd ** -0.5
    assert c % 128 == 0 and n % 512 == 0 and h % 2 == 0 and hd == 64
    CCH = c // 128      # contraction chunks over channels (6)
    NHP = h // 2        # head pairs (6)
    NQB = n // 512      # q-blocks per sequence (2)
    NKT = n // 128      # 128-wide k/token tiles per sequence (8)
    NXH = n // 512      # 512-wide x tiles per sequence (2)
    VW = hd + 1         # v' width per head (ones col at hd)
    PH = c // 2         # proj/v free-dim half (384) <= 1 PSUM bank

    FP32 = mybir.dt.float32
    SD = mybir.dt.bfloat16 if compute == "bf16" else FP32  # storage dtype

    def mm(ap):
        return ap.bitcast(mybir.dt.float32r) if compute == "f32r" else ap

    nc = bacc.Bacc("TRN2", target_bir_lowering=False, debug=False,
                   num_devices=NCORES)

    xT_d = nc.dram_tensor("xT", [c, t], SD, kind="ExternalInput").ap()
    wqkv_d = nc.dram_tensor("w_qkv", [c, 3 * c], SD, kind="ExternalInput").ap()
    wproj_d = nc.dram_tensor("w_proj", [c, c], SD, kind="ExternalInput").ap()
    bias_d = nc.dram_tensor("bias", [128, c], FP32, kind="ExternalInput").ap()
    out_d = nc.dram_tensor("out", [t, c], FP32, kind="ExternalOutput").ap()

    Exp = mybir.ActivationFunctionType.Exp

    with tile.TileContext(nc) as tc, ExitStack() as ctx:
        consts = ctx.enter_context(tc.tile_pool(name="consts", bufs=1))
        xp = ctx.enter_context(tc.tile_pool(name="xp", bufs=2))
        qkp = ctx.enter_context(tc.tile_pool(name="qkp", bufs=2))
        vp = ctx.enter_context(tc.tile_pool(name="vp", bufs=2))
        ep = ctx.enter_context(tc.tile_pool(name="ep", bufs=4))
        aop = ctx.enter_context(tc.tile_pool(name="aop", bufs=2))
        smp = ctx.enter_context(tc.tile_pool(name="smp", bufs=2))
        yp = ctx.enter_context(tc.tile_pool(name="yp", bufs=4))
        # PSUM: 16KB/partition total. s: 2x4KB, u: 2x2KB, p: 2x2KB.
        ps_s = ctx.enter_context(tc.tile_pool(name="ps_s", bufs=2, space="PSUM"))
        ps_u = ctx.enter_context(tc.tile_pool(name="ps_u", bufs=2, space="PSUM"))
        ps_p = ctx.enter_context(tc.tile_pool(name="ps_p", bufs=2, space="PSUM"))

        # ---------------- DMA loads (ordered for the pipeline ramp) -------
        # The load path is one serialized queue at ~320GB/s, so order by
        # first-use: wv + x(b0,xh0) (v-phase), then just the two 128-col
        # slivers of wqk that hp0 needs, then x(b0,xh1), then the rest.
        # Dependency tracking is tile-granular and every DMA transfer costs
        # ~600ns on the serialized load queue, so: hp0's qk weights get their
        # own small merged tiles (no false dep on the bulk wqk load), and the
        # per-cc x halves merge into ONE transfer per (batch, half) via a
        # rearranged access pattern.
        # Three parallel load queues (Sync, Scalar, GpSimd): weights on Sync,
        # x(b0) on Scalar (idle until the first exp), hp0's qk slivers on
        # GpSimd. Each queue serializes at ~320GB/s with ~600ns/transfer
        # minimum, so the head-critical bytes split across all three.
        def merged_x(b, xh, half, eng):
            xt = xp.tile([128, 3 * 512], SD, tag=f"xm{b}_{xh}_{half}",
                         name=f"x_b{b}h{xh}q{half}")
            c0 = half * 3 * 128
            src = xT_d[c0:c0 + 3 * 128,
                       b * n + xh * 512:b * n + (xh + 1) * 512]
            eng.dma_start(
                out=xt.rearrange("p (cc t) -> p cc t", cc=3),
                in_=src.rearrange("(cc p) t -> p cc t", cc=3))
            return xt

        # PE warmup: ~8 throwaway matmuls ramp the PE clock to full speed
        # before the first real group issues.
        wrm = consts.tile([128, 512], SD, tag="warm")
        nc.vector.memset(wrm, 0.0)
        xm = {}
        wq0_all = consts.tile([128, CCH * 128], SD, tag="wq0")
        nc.gpsimd.dma_start(
            out=wq0_all.rearrange("p (cc w) -> p cc w", cc=CCH),
            in_=wqkv_d[0:c, 0:128].rearrange("(cc p) w -> p cc w", cc=CCH))
        wk0_all = consts.tile([128, CCH * 128], SD, tag="wk0")
        nc.gpsimd.dma_start(
            out=wk0_all.rearrange("p (cc w) -> p cc w", cc=CCH),
            in_=wqkv_d[0:c, c:c + 128].rearrange("(cc p) w -> p cc w", cc=CCH))
        for xh in range(NXH):
            for half in range(2):
                xm[(0, xh, half)] = merged_x(0, xh, half, nc.scalar)
        wv_sb = []
        wqk_sb = []
        for cc in range(CCH):
            wv = consts.tile([128, c], SD, tag=f"wv{cc}")
            nc.sync.dma_start(out=wv, in_=wqkv_d[cc * 128:(cc + 1) * 128,
                                                 2 * c:3 * c])
            wv_sb.append(wv)

        def x_ap(b, cc, xh):
            return xm[(b, xh, cc // 3)][:, (cc % 3) * 512:(cc % 3 + 1) * 512]

        for _ in range(9):
            wps = ps_s.tile([128, 512], FP32, tag="s", name="warmup")
            nc.tensor.matmul(wps, lhsT=mm(wrm[:, 0:128]), rhs=mm(wrm),
                             start=True, stop=True)

        # bulk wqk (used from hp1 on), then x(b1), wproj, bias (mid-program).
        for cc in range(CCH):
            w1 = consts.tile([128, 2 * c], SD, tag=f"wqkv{cc}")
            nc.sync.dma_start(out=w1, in_=wqkv_d[cc * 128:(cc + 1) * 128,
                                                 0:2 * c])
            wqk_sb.append(w1)
        for xh in range(NXH):
            for half in range(2):
                xm[(1, xh, half)] = merged_x(1, xh, half, nc.sync)
        wproj_sb = []
        for cc in range(CCH):
            w2 = consts.tile([128, c], SD, tag=f"wproj{cc}")
            nc.sync.dma_start(out=w2, in_=wproj_d[cc * 128:(cc + 1) * 128, :])
            wproj_sb.append(w2)
        bias_sb = consts.tile([128, c], FP32, tag="bias")
        nc.sync.dma_start(out=bias_sb, in_=bias_d)

        # ---------------- group emitters ---------------------------------
        # Each emits 6 accumulating matmuls into a PSUM tile from pool/tag,
        # then a high-priority DVE evacuation.

        v_all = [[None] * NKT for _ in range(bl)]

        def v_tile_of(b, tt):
            if v_all[b][tt] is None:
                vt = vp.tile([128, h * VW], SD, tag=f"v{tt}", name=f"v_b{b}t{tt}")
                ones_view = vt[:, :].rearrange("p (hh w) -> p hh w", hh=h)[:, :, hd:hd + 1]
                nc.gpsimd.memset(ones_view, 1.0)
                v_all[b][tt] = vt
            return v_all[b][tt]

        HCC = CCH // 2   # matmuls per filler half-group

        def v_parts(b, tt, half, pool, ptag):
            st = {}

            def part(lo, hi):
                def go():
                    vt = v_tile_of(b, tt)
                    if lo == 0:
                        st["ps"] = pool.tile([128, PH], FP32, tag=ptag,
                                             name=f"vps_b{b}t{tt}f{half}")
                    ps = st["ps"]
                    xh, tl = tt // 4, tt % 4
                    for cc in range(lo, hi):
                        nc.tensor.matmul(
                            ps,
                            lhsT=mm(x_ap(b, cc, xh)[:, tl * 128:(tl + 1) * 128]),
                            rhs=mm(wv_sb[cc][:, half * PH:(half + 1) * PH]),
                            start=(cc == 0), stop=(cc == CCH - 1))
                    if hi == CCH:
                        nheads = PH // hd
                        dst = vt[:, half * nheads * VW:(half + 1) * nheads * VW
                                 ].rearrange("p (hh w) -> p hh w",
                                             hh=nheads)[:, :, 0:hd]
                        srcv = ps[:].rearrange("p (hh w) -> p hh w", hh=nheads)
                        with tc.high_priority(offset=300):
                            nc.vector.tensor_copy(dst, srcv)
                return go
            return [part(0, HCC), part(HCC, CCH)]

        def emit_v_group(b, tt, half, pool, ptag):
            for p in v_parts(b, tt, half, pool, ptag):
                p()

        qk_tiles = {}

        def qk_dst(b, hp, which, qn):
            key = (b, hp, which, qn)
            if key not in qk_tiles:
                qk_tiles[key] = qkp.tile([128, 512], SD, tag=f"{which}{qn}",
                                         name=f"{which}{qn}_b{b}hp{hp}")
            return qk_tiles[key]

        def qk_parts(b, hp, which, qn, pool, ptag):
            st = {}

            def part(lo, hi):
                def go():
                    dst = qk_dst(b, hp, which, qn)
                    if hp == 0:
                        w0 = wq0_all if which == "qt" else wk0_all
                        w_of = lambda cc: w0[:, cc * 128:(cc + 1) * 128]
                    else:
                        fbase = hp * 128 if which == "qt" else c + hp * 128
                        w_of = lambda cc: wqk_sb[cc][:, fbase:fbase + 128]
                    if lo == 0:
                        st["ps"] = pool.tile([128, 512], FP32, tag=ptag,
                                             name=f"qkps_{which}_b{b}hp{hp}q{qn}")
                    ps = st["ps"]
                    for cc in range(lo, hi):
                        nc.tensor.matmul(
                            ps,
                            lhsT=mm(w_of(cc)),
                            rhs=mm(x_ap(b, cc, qn)),
                            start=(cc == 0), stop=(cc == CCH - 1))
                    if hi == CCH:
                        with tc.high_priority(offset=300):
                            nc.vector.tensor_copy(dst, ps)
                return go
            return [part(0, HCC), part(HCC, CCH)]

        def emit_qk_group(b, hp, which, qn, pool, ptag):
            for p in qk_parts(b, hp, which, qn, pool, ptag):
                p()

        ao_tiles = {}

        def out_parts(b, tt, half, pool, ptag):
            st = {}

            def part(lo, hi):
                def go():
                    if lo == 0:
                        st["ps"] = pool.tile([128, PH], FP32, tag=ptag,
                                             name=f"yps_b{b}t{tt}f{half}")
                    ps = st["ps"]
                    for cc in range(lo, hi):
                        nc.tensor.matmul(
                            ps,
                            lhsT=mm(ao_tiles[(b, cc)][:, tt * 128:(tt + 1) * 128]),
                            rhs=mm(wproj_sb[cc][:, half * PH:(half + 1) * PH]),
                            start=(cc == 0), stop=(cc == CCH - 1))
                    if hi == CCH:
                        yt = yp.tile([128, PH], FP32, tag="y",
                                     name=f"y_b{b}t{tt}f{half}")
                        with tc.high_priority(offset=300):
                            nc.vector.tensor_add(
                                yt, ps, bias_sb[:, half * PH:(half + 1) * PH])
                        nc.sync.dma_start(
                            out=out_d[b * n + tt * 128:b * n + (tt + 1) * 128,
                                      half * PH:(half + 1) * PH],
                            in_=yt)
                return go
            return [part(0, HCC), part(HCC, CCH)]

        def emit_out_group(b, tt, half, pool, ptag):
            for p in out_parts(b, tt, half, pool, ptag):
                p()

        # ---------------- filler pump ------------------------------------
        fillers = deque()

        def pump(k=1):
            for _ in range(k):
                if fillers:
                    fillers.popleft()()

        # ---------------- head phase -------------------------------------
        # Just enough to start attention: v(b0, tt0) both halves + qk(b0,0).
        # Rotate over the (still idle) u and p slots for double buffering.
        head_rot = [("u", ps_u), ("p", ps_p)]
        head_groups = []
        for qn in range(NQB):
            for which in ("qt", "kt"):
                head_groups.append(
                    (lambda which=which, qn=qn: lambda pool, ptag:
                     emit_qk_group(0, 0, which, qn, pool, ptag))())
        for i, g in enumerate(head_groups):
            ptag, pool = head_rot[i % 2]
            g(pool, ptag)

        # ---------------- window filler schedule --------------------------
        # Each entry is one pump event: either a whole v group (JIT, hp0) or
        # a 3-matmul half-group. 16 pump slots per window (one per kt-step).
        # v halves: half 0 feeds head pairs 0..2, half 1 feeds 3..5.
        def qk_f(b, hp):
            out = []
            for qn in range(NQB):
                for which in ("qt", "kt"):
                    out.extend(qk_parts(b, hp, which, qn, ps_p, "p"))
            return out

        def v_f(b, tts, half):
            # whole groups: one LDW stall amortized over 6 matmuls
            return [(lambda tt=tt: emit_v_group(b, tt, half, ps_p, "p"))
                    for tt in tts]

        def window_fillers(b, hp):
            if b == 0:
                if hp == 0:
                    # JIT v(b0) half-0 as whole groups (tt0 both halves
                    # first); with pump rate 2 early, v tt_k lands just
                    # before U(kt=k) needs it.
                    return [lambda: emit_v_group(0, 0, 0, ps_p, "p"),
                            lambda: emit_v_group(0, 0, 1, ps_p, "p")] + \
                           [(lambda tt=tt: emit_v_group(0, tt, 0, ps_p, "p"))
                            for tt in range(1, NKT)] + qk_f(0, 1)
                if hp == 1:
                    return qk_f(0, 2) + v_f(0, range(0, 4), 1)
                if hp == 2:
                    return qk_f(0, 3) + v_f(0, range(4, NKT), 1)
                if hp == 3:
                    return qk_f(0, 4) + v_f(1, range(0, 4), 0)
                if hp == 4:
                    return qk_f(0, 5) + v_f(1, range(4, NKT), 0)
                return qk_f(1, 0)
            else:
                all_out = [(tt, half) for tt in range(NKT) for half in range(2)]
                out = []
                if hp < NHP - 1:
                    out += qk_f(1, hp + 1)
                if hp == 0:
                    out += v_f(1, range(0, 4), 1)
                elif hp == 1:
                    out += v_f(1, range(4, NKT), 1)
                else:
                    for tt, half in all_out[(hp - 2) * 4:(hp - 1) * 4]:
                        out.append(lambda tt=tt, half=half:
                                   emit_out_group(0, tt, half, ps_p, "p"))
                return out

        # ---------------- attention: global software pipeline -------------
        steps = [(b, hp, qblk, kt)
                 for b in range(bl)
                 for hp in range(NHP)
                 for qblk in range(NQB)
                 for kt in range(NKT)]

        u_ps = {}     # (qblk % 2, head) -> psum accum tile (ring by alloc)
        et_by_step = {}
        ao_cur = {}   # (b, hp) -> ao tile

        def emit_S(i):
            b, hp, qblk, kt = steps[i]
            if qblk == 0 and kt == 0:
                ao_cur[(b, hp)] = aop.tile([128, n], SD, tag=f"ao{hp}",
                                           name=f"ao_b{b}hp{hp}")
                fillers.extend(window_fillers(b, hp))
            qt_t = qk_dst(b, hp, "qt", qblk)
            kt_t = qk_dst(b, hp, "kt", kt // 4)
            ko = (kt % 4) * 128
            sps = ps_s.tile([128, 2 * 512], FP32, tag="s",
                            name=f"s_b{b}hp{hp}q{qblk}k{kt}")
            for head in range(2):
                p0 = head * 64
                nc.tensor.matmul(
                    sps[:, head * 512:(head + 1) * 512],
                    lhsT=mm(kt_t[p0:p0 + 64, ko:ko + 128]),
                    rhs=mm(qt_t[p0:p0 + 64, :]),
                    start=True, stop=True)
            et = ep.tile([128, 2 * 512], SD, tag="e",
                         name=f"e_b{b}hp{hp}q{qblk}k{kt}")
            nc.scalar.activation(et, sps, Exp, scale=scale)
            et_by_step[i] = et

        def emit_U(i):
            b, hp, qblk, kt = steps[i]
            et = et_by_step.pop(i)
            base = i - kt
            if kt == 0:
                for head in range(2):
                    u_ps[(base, head)] = ps_u.tile(
                        [VW, 512], FP32, tag="u",
                        name=f"u_b{b}hp{hp}q{qblk}h{head}")
            for head in range(2):
                hh = 2 * hp + head
                nc.tensor.matmul(
                    u_ps[(base, head)],
                    lhsT=mm(v_all[b][kt][:, hh * VW:hh * VW + VW]),
                    rhs=mm(et[:, head * 512:(head + 1) * 512]),
                    start=(kt == 0), stop=(kt == NKT - 1))
            if kt == NKT - 1:
                emit_normalize(i, base)

        def emit_normalize(i, base):
            b, hp, qblk, kt = steps[i]
            ao = ao_cur[(b, hp)]
            qs = slice(qblk * 512, (qblk + 1) * 512)
            # the whole chain runs at high priority so next-window fillers
            # can't starve it on DVE/GpSimd (its copies gate U-bank release);
            # the last window gates the whole tail: jump even further
            last = (b == bl - 1 and hp == NHP - 1)
            stk = tc.high_priority(offset=3000 if last else 300)
            stk.__enter__()
            usbs = {}
            for head in (1, 0):
                usb = smp.tile([VW, 512], FP32, tag=f"usb{head}",
                               name=f"usb_b{b}hp{hp}q{qblk}h{head}")
                # gates the U-accumulator bank release: jump the DVE queue
                # and run both evacuations before either head's chain
                with tc.high_priority(offset=300):
                    nc.vector.tensor_copy(usb, u_ps.pop((base, head)))
                usbs[head] = usb
            for head in (1, 0):
                usb = usbs[head]
                # Z row -> partition 0 (DMA), broadcast to 64 partitions
                # (gpsimd), then reciprocal on the full-width tile (the
                # custom DVE op mis-executes on 1-partition slices at
                # base partition != 0).
                z1 = smp.tile([1, 512], FP32, tag=f"z1{head}", bufs=1,
                              name=f"z1_b{b}hp{hp}q{qblk}h{head}")
                nc.gpsimd.dma_start(out=z1, in_=usb[hd:hd + 1, :])
                rb = smp.tile([64, 512], FP32, tag=f"rb{head}",
                              name=f"rb_b{b}hp{hp}q{qblk}h{head}")
                nc.gpsimd.partition_broadcast(rb, z1)
                nc.vector.reciprocal_approx_fast(rb, rb)
                if head == 0:
                    nc.vector.tensor_mul(ao[0:64, qs], usb[0:hd, :], rb)
                else:
                    sc = smp.tile([64, 512], SD, tag="sc",
                                  name=f"sc_b{b}hp{hp}q{qblk}")
                    nc.vector.tensor_mul(sc, usb[0:hd, :], rb)
                    nc.gpsimd.dma_start(out=ao[64:128, qs], in_=sc)
            stk.__exit__(None, None, None)
            if qblk == NQB - 1:
                ao_tiles[(b, hp)] = ao

        for i in range(len(steps) + 1):
            if i < len(steps):
                emit_S(i)
            if i > 0:
                if steps[i - 1][:2] == (0, 0):
                    # hp0: v-JIT fillers must be EMITTED before the U that
                    # reads them (write-before-read in program order), and
                    # drained fast (v tt_k lands before U(kt=k) issues)
                    pump(2 if i <= 6 else 1)
                    emit_U(i - 1)
                else:
                    # U right after the S-pair: its 65-col LDW eats the
                    # post-pair weight-load stall instead of a 128-col one.
                    emit_U(i - 1)
                    pump(1)

        # ---------------- tail: out-proj(b1) ------------------------------
        tail_rot = [("p", ps_p), ("s", ps_s)]
        gi = 0
        for tt in range(NKT):
            for half in range(2):
                ptag, pool = tail_rot[gi % 2]
                emit_out_group(1, tt, half, pool, ptag)
                gi += 1
        # drain any leftover fillers (shouldn't be any)
        while fillers:
            fillers.popleft()()

    nc.compile()
    return nc


_NC_CACHE = {}


def _get_nc(compute=COMPUTE):
    if compute not in _NC_CACHE:
        _NC_CACHE[compute] = build_attention_nc(compute)
    return _NC_CACHE[compute]


def make_in_maps(x, W_qkv, W_proj, b_proj, compute=None):
    compute = compute or COMPUTE
    if compute == "bf16":
        import ml_dtypes
        sd = ml_dtypes.bfloat16
    else:
        sd = np.float32
    x = np.asarray(x, dtype=np.float32)
    W_qkv = np.ascontiguousarray(np.asarray(W_qkv, dtype=np.float32)).astype(sd)
    W_proj = np.ascontiguousarray(np.asarray(W_proj, dtype=np.float32)).astype(sd)
    bias = np.ascontiguousarray(
        np.broadcast_to(np.asarray(b_proj, dtype=np.float32), (128, C)))
    in_maps = []
    for i in range(NCORES):
        shard = x[i * BL:(i + 1) * BL]                      # [BL, N, C]
        xT = np.ascontiguousarray(shard.transpose(2, 0, 1).reshape(C, T)).astype(sd)
        in_maps.append({"xT": xT, "w_qkv": W_qkv, "w_proj": W_proj,
                        "bias": bias})
    return in_maps


def kernel(x, W_qkv, W_proj, b_proj):
    from concourse.bass_utils import run_bass_kernel_spmd

    nc = _get_nc()
    in_maps = make_in_maps(x, W_qkv, W_proj, b_proj)
    res = run_bass_kernel_spmd(nc, in_maps, core_ids=list(range(NCORES)))
    outs = [res.results[i]["out"].reshape(BL, N, C) for i in range(NCORES)]
    return np.concatenate(outs, axis=0).astype(np.float32)


if __name__ == "__main__":
    nc = build_attention_nc()
    print("built ok")


# revision 34
# speedup vs baseline: 1.0763x; 1.0441x over previous
"""Trainium2 Bass kernel: multi-head self-attention block (B=16, N=1024, C=768, H=12).

Data-parallel over batch: 8 NeuronCores x 2 batches each, no collectives.

Dataflow (per core, all-transposed activations; no on-chip transposes):
  host: xT = x_shard^T                                  [C, T]
  qkT  = W_qkv[:, :2C]^T-tiles @ xT   (per batch)       [2C, N]   (q^T | k^T)
  v'   = xT-tiles^T @ W_qkv[:, 2C:]  (+ ones col/head)  [N, H*(HD+1)]
  S^T  = k^T-slices^T @ q^T  (head pair packed in one   [128, 1024]
         2-bank PSUM tile: head A cols 0:512, B 512:)
  E    = exp(SCALE * S^T)     (ONE ScalarE op per step)
  U'   = v'^T @ E  (accum over k; row HD = softmax Z)   [HD+1, 512]
  aoT  = U'[:HD] * (1/Z broadcast)                      [C, N]
  y    = aoT-tiles^T @ W_proj + b                       [N, C]

Scheduling: one global software pipeline over (batch, head-pair, q-block,
k-tile) steps.  S(i+1) is emitted before U(i) so the PE never sits on the
exp latency; all projection work (v-phase, qk projections, out-proj of
batch 0) is chopped into 6-matmul "filler" groups pumped one-per-step into
2 spare PSUM banks, hiding it inside the Scalar-paced attention windows.
PSUM: S-ring 2x[128,1024] (8KB) + U-accum 2x[65,512] (4KB) + filler
2x[128,512] (4KB) = 16KB exactly.
"""

import sys
from collections import deque

for _p in ("/opt/trn_rl_repo", "/opt/pypackages"):
    if _p not in sys.path:
        sys.path.append(_p)

import numpy as np

B, N, C, H = 16, 1024, 768, 12
HD = C // H            # 64
SCALE = HD ** -0.5
NCORES = 8
BL = B // NCORES       # 2 batches per core
T = BL * N             # 2048 tokens per core

COMPUTE = "bf16"       # "bf16" | "f32" | "f32r"


def build_attention_nc(compute=COMPUTE, bl=BL, n=N, c=C, h=H):
    import concourse.bass as bass
    import concourse.tile as tile
    from concourse import bacc, mybir
    from contextlib import ExitStack

    hd = c // h
    t = bl * n
    scale = hd ** -0.5
    assert c % 128 == 0 and n % 512 == 0 and h % 2 == 0 and hd == 64
    CCH = c // 128      # contraction chunks over channels (6)
    NHP = h // 2        # head pairs (6)
    NQB = n // 512      # q-blocks per sequence (2)
    NKT = n // 128      # 128-wide k/token tiles per sequence (8)
    NXH = n // 512      # 512-wide x tiles per sequence (2)
    VW = hd + 1         # v' width per head (ones col at hd)
    PH = c // 2         # proj/v free-dim half (384) <= 1 PSUM bank

    FP32 = mybir.dt.float32
    SD = mybir.dt.bfloat16 if compute == "bf16" else FP32  # storage dtype

    def mm(ap):
        return ap.bitcast(mybir.dt.float32r) if compute == "f32r" else ap

    nc = bacc.Bacc("TRN2", target_bir_lowering=False, debug=False,
                   num_devices=NCORES)

    xT_d = nc.dram_tensor("xT", [c, t], SD, kind="ExternalInput").ap()
    wqkv_d = nc.dram_tensor("w_qkv", [c, 3 * c], SD, kind="ExternalInput").ap()
    wproj_d = nc.dram_tensor("w_proj", [c, c], SD, kind="ExternalInput").ap()
    bias_d = nc.dram_tensor("bias", [128, c], FP32, kind="ExternalInput").ap()
    out_d = nc.dram_tensor("out", [t, c], FP32, kind="ExternalOutput").ap()

    Exp = mybir.ActivationFunctionType.Exp

    with tile.TileContext(nc) as tc, ExitStack() as ctx:
        consts = ctx.enter_context(tc.tile_pool(name="consts", bufs=1))
        xp = ctx.enter_context(tc.tile_pool(name="xp", bufs=2))
        qkp = ctx.enter_context(tc.tile_pool(name="qkp", bufs=2))
        vp = ctx.enter_context(tc.tile_pool(name="vp", bufs=2))
        ep = ctx.enter_context(tc.tile_pool(name="ep", bufs=4))
        aop = ctx.enter_context(tc.tile_pool(name="aop", bufs=2))
        smp = ctx.enter_context(tc.tile_pool(name="smp", bufs=2))
        yp = ctx.enter_context(tc.tile_pool(name="yp", bufs=4))
        # PSUM: 16KB/partition total. s: 2x4KB, u: 2x2KB, p: 2x2KB.
        ps_s = ctx.enter_context(tc.tile_pool(name="ps_s", bufs=2, space="PSUM"))
        ps_u = ctx.enter_context(tc.tile_pool(name="ps_u", bufs=2, space="PSUM"))
        ps_p = ctx.enter_context(tc.tile_pool(name="ps_p", bufs=2, space="PSUM"))

        # ---------------- DMA loads (ordered for the pipeline ramp) -------
        # The load path is one serialized queue at ~320GB/s, so order by
        # first-use: wv + x(b0,xh0) (v-phase), then just the two 128-col
        # slivers of wqk that hp0 needs, then x(b0,xh1), then the rest.
        # Dependency tracking is tile-granular and every DMA transfer costs
        # ~600ns on the serialized load queue, so: hp0's qk weights get their
        # own small merged tiles (no false dep on the bulk wqk load), and the
        # per-cc x halves merge into ONE transfer per (batch, half) via a
        # rearranged access pattern.
        # Three parallel load queues (Sync, Scalar, GpSimd): weights on Sync,
        # x(b0) on Scalar (idle until the first exp), hp0's qk slivers on
        # GpSimd. Each queue serializes at ~320GB/s with ~600ns/transfer
        # minimum, so the head-critical bytes split across all three.
        def merged_x(b, xh, half, eng):
            xt = xp.tile([128, 3 * 512], SD, tag=f"xm{b}_{xh}_{half}",
                         name=f"x_b{b}h{xh}q{half}")
            c0 = half * 3 * 128
            src = xT_d[c0:c0 + 3 * 128,
                       b * n + xh * 512:b * n + (xh + 1) * 512]
            eng.dma_start(
                out=xt.rearrange("p (cc t) -> p cc t", cc=3),
                in_=src.rearrange("(cc p) t -> p cc t", cc=3))
            return xt

        # PE warmup: ~8 throwaway matmuls ramp the PE clock to full speed
        # before the first real group issues.
        wrm = consts.tile([128, 512], SD, tag="warm")
        nc.vector.memset(wrm, 0.0)
        xm = {}
        wq0_all = consts.tile([128, CCH * 128], SD, tag="wq0")
        nc.gpsimd.dma_start(
            out=wq0_all.rearrange("p (cc w) -> p cc w", cc=CCH),
            in_=wqkv_d[0:c, 0:128].rearrange("(cc p) w -> p cc w", cc=CCH))
        wk0_all = consts.tile([128, CCH * 128], SD, tag="wk0")
        nc.gpsimd.dma_start(
            out=wk0_all.rearrange("p (cc w) -> p cc w", cc=CCH),
            in_=wqkv_d[0:c, c:c + 128].rearrange("(cc p) w -> p cc w", cc=CCH))
        for xh in range(NXH):
            for half in range(2):
                xm[(0, xh, half)] = merged_x(0, xh, half, nc.scalar)
        wv_sb = []
        wqk_sb = []
        for cc in range(CCH):
            wv = consts.tile([128, c], SD, tag=f"wv{cc}")
            nc.sync.dma_start(out=wv, in_=wqkv_d[cc * 128:(cc + 1) * 128,
                                                 2 * c:3 * c])
            wv_sb.append(wv)

        def x_ap(b, cc, xh):
            return xm[(b, xh, cc // 3)][:, (cc % 3) * 512:(cc % 3 + 1) * 512]

        for _ in range(9):
            wps = ps_s.tile([128, 512], FP32, tag="s", name="warmup")
            nc.tensor.matmul(wps, lhsT=mm(wrm[:, 0:128]), rhs=mm(wrm),
                             start=True, stop=True)

        # bulk wqk (used from hp1 on), then x(b1), wproj, bias (mid-program).
        for cc in range(CCH):
            w1 = consts.tile([128, 2 * c], SD, tag=f"wqkv{cc}")
            nc.sync.dma_start(out=w1, in_=wqkv_d[cc * 128:(cc + 1) * 128,
                                                 0:2 * c])
            wqk_sb.append(w1)
        for xh in range(NXH):
            for half in range(2):
                xm[(1, xh, half)] = merged_x(1, xh, half, nc.sync)
        wproj_sb = []
        for cc in range(CCH):
            w2 = consts.tile([128, c], SD, tag=f"wproj{cc}")
            nc.sync.dma_start(out=w2, in_=wproj_d[cc * 128:(cc + 1) * 128, :])
            wproj_sb.append(w2)
        bias_sb = consts.tile([128, c], FP32, tag="bias")
        nc.sync.dma_start(out=bias_sb, in_=bias_d)

        # ---------------- group emitters ---------------------------------
        # Each emits 6 accumulating matmuls into a PSUM tile from pool/tag,
        # then a high-priority DVE evacuation.

        v_all = [[None] * NKT for _ in range(bl)]

        def v_tile_of(b, tt):
            if v_all[b][tt] is None:
                vt = vp.tile([128, h * VW], SD, tag=f"v{tt}", name=f"v_b{b}t{tt}")
                ones_view = vt[:, :].rearrange("p (hh w) -> p hh w", hh=h)[:, :, hd:hd + 1]
                nc.gpsimd.memset(ones_view, 1.0)
                v_all[b][tt] = vt
            return v_all[b][tt]

        HCC = CCH // 2   # matmuls per filler half-group

        def v_parts(b, tt, half, pool, ptag):
            st = {}

            def part(lo, hi):
                def go():
                    vt = v_tile_of(b, tt)
                    if lo == 0:
                        st["ps"] = pool.tile([128, PH], FP32, tag=ptag,
                                             name=f"vps_b{b}t{tt}f{half}")
                    ps = st["ps"]
                    xh, tl = tt // 4, tt % 4
                    for cc in range(lo, hi):
                        nc.tensor.matmul(
                            ps,
                            lhsT=mm(x_ap(b, cc, xh)[:, tl * 128:(tl + 1) * 128]),
                            rhs=mm(wv_sb[cc][:, half * PH:(half + 1) * PH]),
                            start=(cc == 0), stop=(cc == CCH - 1))
                    if hi == CCH:
                        nheads = PH // hd
                        dst = vt[:, half * nheads * VW:(half + 1) * nheads * VW
                                 ].rearrange("p (hh w) -> p hh w",
                                             hh=nheads)[:, :, 0:hd]
                        srcv = ps[:].rearrange("p (hh w) -> p hh w", hh=nheads)
                        with tc.high_priority(offset=300):
                            nc.vector.tensor_copy(dst, srcv)
                return go
            return [part(0, HCC), part(HCC, CCH)]

        def emit_v_group(b, tt, half, pool, ptag):
            for p in v_parts(b, tt, half, pool, ptag):
                p()

        qk_tiles = {}

        def qk_dst(b, hp, which, qn):
            key = (b, hp, which, qn)
            if key not in qk_tiles:
                qk_tiles[key] = qkp.tile([128, 512], SD, tag=f"{which}{qn}",
                                         name=f"{which}{qn}_b{b}hp{hp}")
            return qk_tiles[key]

        def qk_parts(b, hp, which, qn, pool, ptag):
            st = {}

            def part(lo, hi):
                def go():
                    dst = qk_dst(b, hp, which, qn)
                    if hp == 0:
                        w0 = wq0_all if which == "qt" else wk0_all
                        w_of = lambda cc: w0[:, cc * 128:(cc + 1) * 128]
                    else:
                        fbase = hp * 128 if which == "qt" else c + hp * 128
                        w_of = lambda cc: wqk_sb[cc][:, fbase:fbase + 128]
                    if lo == 0:
                        st["ps"] = pool.tile([128, 512], FP32, tag=ptag,
                                             name=f"qkps_{which}_b{b}hp{hp}q{qn}")
                    ps = st["ps"]
                    for cc in range(lo, hi):
                        nc.tensor.matmul(
                            ps,
                            lhsT=mm(w_of(cc)),
                            rhs=mm(x_ap(b, cc, qn)),
                            start=(cc == 0), stop=(cc == CCH - 1))
                    if hi == CCH:
                        with tc.high_priority(offset=300):
                            nc.vector.tensor_copy(dst, ps)
                return go
            return [part(0, HCC), part(HCC, CCH)]

        def emit_qk_group(b, hp, which, qn, pool, ptag):
            for p in qk_parts(b, hp, which, qn, pool, ptag):
                p()

        ao_tiles = {}

        def out_parts(b, tt, half, pool, ptag):
            st = {}

            def part(lo, hi):
                def go():
                    if lo == 0:
                        st["ps"] = pool.tile([128, PH], FP32, tag=ptag,
                                             name=f"yps_b{b}t{tt}f{half}")
                    ps = st["ps"]
                    for cc in range(lo, hi):
                        nc.tensor.matmul(
                            ps,
                            lhsT=mm(ao_tiles[(b, cc)][:, tt * 128:(tt + 1) * 128]),
                            rhs=mm(wproj_sb[cc][:, half * PH:(half + 1) * PH]),
                            start=(cc == 0), stop=(cc == CCH - 1))
                    if hi == CCH:
                        yt = yp.tile([128, PH], FP32, tag="y",
                                     name=f"y_b{b}t{tt}f{half}")
                        with tc.high_priority(offset=300):
                            nc.vector.tensor_add(
                                yt, ps, bias_sb[:, half * PH:(half + 1) * PH])
                        nc.sync.dma_start(
                            out=out_d[b * n + tt * 128:b * n + (tt + 1) * 128,
                                      half * PH:(half + 1) * PH],
                            in_=yt)
                return go
            return [part(0, HCC), part(HCC, CCH)]

        def emit_out_group(b, tt, half, pool, ptag):
            for p in out_parts(b, tt, half, pool, ptag):
                p()

        # ---------------- filler pump ------------------------------------
        fillers = deque()

        def pump(k=1):
            for _ in range(k):
                if fillers:
                    fillers.popleft()()

        # ---------------- head phase -------------------------------------
        # Just enough to start attention: v(b0, tt0) both halves + qk(b0,0).
        # Rotate over the (still idle) u and p slots for double buffering.
        head_rot = [("u", ps_u), ("p", ps_p)]
        head_groups = []
        for qn in range(NQB):
            for which in ("qt", "kt"):
                head_groups.append(
                    (lambda which=which, qn=qn: lambda pool, ptag:
                     emit_qk_group(0, 0, which, qn, pool, ptag))())
        for i, g in enumerate(head_groups):
            ptag, pool = head_rot[i % 2]
            g(pool, ptag)

        # ---------------- window filler schedule --------------------------
        # Each entry is one pump event: either a whole v group (JIT, hp0) or
        # a 3-matmul half-group. 16 pump slots per window (one per kt-step).
        # v halves: half 0 feeds head pairs 0..2, half 1 feeds 3..5.
        def qk_f(b, hp):
            out = []
            for qn in range(NQB):
                for which in ("qt", "kt"):
                    out.extend(qk_parts(b, hp, which, qn, ps_p, "p"))
            return out

        def v_f(b, tts, half):
            # whole groups: one LDW stall amortized over 6 matmuls
            return [(lambda tt=tt: emit_v_group(b, tt, half, ps_p, "p"))
                    for tt in tts]

        def window_fillers(b, hp):
            if b == 0:
                if hp == 0:
                    # JIT v(b0) half-0 as whole groups (tt0 both halves
                    # first); with pump rate 2 early, v tt_k lands just
                    # before U(kt=k) needs it.
                    return [lambda: emit_v_group(0, 0, 0, ps_p, "p"),
                            lambda: emit_v_group(0, 0, 1, ps_p, "p")] + \
                           [(lambda tt=tt: emit_v_group(0, tt, 0, ps_p, "p"))
                            for tt in range(1, NKT)] + qk_f(0, 1)
                if hp == 1:
                    return qk_f(0, 2) + v_f(0, range(0, 4), 1)
                if hp == 2:
                    return qk_f(0, 3) + v_f(0, range(4, NKT), 1)
                if hp == 3:
                    return qk_f(0, 4) + v_f(1, range(0, 4), 0)
                if hp == 4:
                    return qk_f(0, 5) + v_f(1, range(4, NKT), 0)
                return qk_f(1, 0)
            else:
                all_out = [(tt, half) for tt in range(NKT) for half in range(2)]
                out = []
                if hp < NHP - 1:
                    out += qk_f(1, hp + 1)
                if hp == 0:
                    out += v_f(1, range(0, 4), 1)
                elif hp == 1:
                    out += v_f(1, range(4, NKT), 1)
                else:
                    for tt, half in all_out[(hp - 2) * 4:(hp - 1) * 4]:
                        out.append(lambda tt=tt, half=half:
                                   emit_out_group(0, tt, half, ps_p, "p"))
                return out

        # ---------------- attention: global software pipeline -------------
        steps = [(b, hp, qblk, kt)
                 for b in range(bl)
                 for hp in range(NHP)
                 for qblk in range(NQB)
                 for kt in range(NKT)]

        u_ps = {}     # (qblk % 2, head) -> psum accum tile (ring by alloc)
        et_by_step = {}
        ao_cur = {}   # (b, hp) -> ao tile

        def emit_S(i):
            b, hp, qblk, kt = steps[i]
            if qblk == 0 and kt == 0:
                ao_cur[(b, hp)] = aop.tile([128, n], SD, tag=f"ao{hp}",
                                           name=f"ao_b{b}hp{hp}")
                fillers.extend(window_fillers(b, hp))
            qt_t = qk_dst(b, hp, "qt", qblk)
            kt_t = qk_dst(b, hp, "kt", kt // 4)
            ko = (kt % 4) * 128
            sps = ps_s.tile([128, 2 * 512], FP32, tag="s",
                            name=f"s_b{b}hp{hp}q{qblk}k{kt}")
            for head in range(2):
                p0 = head * 64
                nc.tensor.matmul(
                    sps[:, head * 512:(head + 1) * 512],
                    lhsT=mm(kt_t[p0:p0 + 64, ko:ko + 128]),
                    rhs=mm(qt_t[p0:p0 + 64, :]),
                    start=True, stop=True)
            et = ep.tile([128, 2 * 512], SD, tag="e",
                         name=f"e_b{b}hp{hp}q{qblk}k{kt}")
            nc.scalar.activation(et, sps, Exp, scale=scale)
            et_by_step[i] = et

        def emit_U(i):
            b, hp, qblk, kt = steps[i]
            et = et_by_step.pop(i)
            base = i - kt
            if kt == 0:
                for head in range(2):
                    u_ps[(base, head)] = ps_u.tile(
                        [VW, 512], FP32, tag="u",
                        name=f"u_b{b}hp{hp}q{qblk}h{head}")
            for head in range(2):
                hh = 2 * hp + head
                nc.tensor.matmul(
                    u_ps[(base, head)],
                    lhsT=mm(v_all[b][kt][:, hh * VW:hh * VW + VW]),
                    rhs=mm(et[:, head * 512:(head + 1) * 512]),
                    start=(kt == 0), stop=(kt == NKT - 1))
            if kt == NKT - 1:
                emit_normalize(i, base)

        def emit_normalize(i, base):
            b, hp, qblk, kt = steps[i]
            ao = ao_cur[(b, hp)]
            qs = slice(qblk * 512, (qblk + 1) * 512)
            # the whole chain runs at high priority so next-window fillers
            # can't starve it on DVE/GpSimd (its copies gate U-bank release);
            # the last window gates the whole tail: jump even further
            last = (b == bl - 1 and hp == NHP - 1)
            stk = tc.high_priority(offset=3000 if last else 300)
            stk.__enter__()
            usbs = {}
            for head in (1, 0):
                usb = smp.tile([VW, 512], FP32, tag=f"usb{head}",
                               name=f"usb_b{b}hp{hp}q{qblk}h{head}")
                # gates the U-accumulator bank release: jump the DVE queue
                # and run both evacuations before either head's chain
                with tc.high_priority(offset=300):
                    nc.vector.tensor_copy(usb, u_ps.pop((base, head)))
                usbs[head] = usb
            for head in (1, 0):
                usb = usbs[head]
                # Z row -> partition 0 (DMA), broadcast to 64 partitions
                # (gpsimd), then reciprocal on the full-width tile (the
                # custom DVE op mis-executes on 1-partition slices at
                # base partition != 0).
                z1 = smp.tile([1, 512], FP32, tag=f"z1{head}", bufs=1,
                              name=f"z1_b{b}hp{hp}q{qblk}h{head}")
                nc.gpsimd.dma_start(out=z1, in_=usb[hd:hd + 1, :])
                rb = smp.tile([64, 512], FP32, tag=f"rb{head}",
                              name=f"rb_b{b}hp{hp}q{qblk}h{head}")
                nc.gpsimd.partition_broadcast(rb, z1)
                nc.vector.reciprocal_approx_fast(rb, rb)
                if head == 0:
                    nc.vector.tensor_mul(ao[0:64, qs], usb[0:hd, :], rb)
                else:
                    sc = smp.tile([64, 512], SD, tag="sc",
                                  name=f"sc_b{b}hp{hp}q{qblk}")
                    nc.vector.tensor_mul(sc, usb[0:hd, :], rb)
                    nc.gpsimd.dma_start(out=ao[64:128, qs], in_=sc)
            stk.__exit__(None, None, None)
            if qblk == NQB - 1:
                ao_tiles[(b, hp)] = ao

        # Steps processed in PAIRS: [S(j),S(j+1)] [U(j-2),U(j-1)] [fillers].
        # Adjacent S-pairs hide each other's 64-row weight loads, so only
        # ONE matmul per block (the first U) eats the post-S-pair LDW
        # stall, instead of one per step.
        NS = len(steps)
        for j in range(0, NS + 2, 2):
            for i in (j, j + 1):
                if i < NS:
                    emit_S(i)
            us = [i for i in (j - 2, j - 1) if 0 <= i < NS]
            if us and steps[us[0]][:2] == (0, 0):
                # hp0: v-JIT fillers must be EMITTED before the U that
                # reads them (write-before-read in program order), and
                # drained fast (v tt_k lands before U(kt=k) issues)
                pump(4 if j <= 6 else 2)
                for i in us:
                    emit_U(i)
            else:
                for i in us:
                    emit_U(i)
                pump(2)

        # ---------------- tail: out-proj(b1) ------------------------------
        tail_rot = [("p", ps_p), ("s", ps_s)]
        gi = 0
        for tt in range(NKT):
            for half in range(2):
                ptag, pool = tail_rot[gi % 2]
                emit_out_group(1, tt, half, pool, ptag)
                gi += 1
        # drain any leftover fillers (shouldn't be any)
        while fillers:
            fillers.popleft()()

    nc.compile()
    return nc


_NC_CACHE = {}


def _get_nc(compute=COMPUTE):
    if compute not in _NC_CACHE:
        _NC_CACHE[compute] = build_attention_nc(compute)
    return _NC_CACHE[compute]


def make_in_maps(x, W_qkv, W_proj, b_proj, compute=None):
    compute = compute or COMPUTE
    if compute == "bf16":
        import ml_dtypes
        sd = ml_dtypes.bfloat16
    else:
        sd = np.float32
    x = np.asarray(x, dtype=np.float32)
    W_qkv = np.ascontiguousarray(np.asarray(W_qkv, dtype=np.float32)).astype(sd)
    W_proj = np.ascontiguousarray(np.asarray(W_proj, dtype=np.float32)).astype(sd)
    bias = np.ascontiguousarray(
        np.broadcast_to(np.asarray(b_proj, dtype=np.float32), (128, C)))
    in_maps = []
    for i in range(NCORES):
        shard = x[i * BL:(i + 1) * BL]                      # [BL, N, C]
        xT = np.ascontiguousarray(shard.transpose(2, 0, 1).reshape(C, T)).astype(sd)
        in_maps.append({"xT": xT, "w_qkv": W_qkv, "w_proj": W_proj,
                        "bias": bias})
    return in_maps


def kernel(x, W_qkv, W_proj, b_proj):
    from concourse.bass_utils import run_bass_kernel_spmd

    nc = _get_nc()
    in_maps = make_in_maps(x, W_qkv, W_proj, b_proj)
    res = run_bass_kernel_spmd(nc, in_maps, core_ids=list(range(NCORES)))
    outs = [res.results[i]["out"].reshape(BL, N, C) for i in range(NCORES)]
    return np.concatenate(outs, axis=0).astype(np.float32)


if __name__ == "__main__":
    nc = build_attention_nc()
    print("built ok")
